# revision 64
# baseline (speedup 1.0000x reference)
"""Two-layer GCN + edge scoring on 8 Trainium2 NeuronCores.

Sharding: nodes row-sharded across cores (6250/core, padded to 6272 = 49
blocks of 128); aggregation edges partitioned by destination core and
grouped by destination block; weights replicated; three device-side
AllGathers move the per-node tables (hs1, hs2, h2) between phases.

Math trick: the GCN symmetric norm dinv[src]*dinv[dst] is separable, so
the gathered tables are pre-scaled by dinv (hs = dinv * (x@W)) and block
outputs post-scaled by dinv — the segment-sum masks stay pure 0/1 and the
scatter-add becomes S^T @ gathered_rows matmuls accumulated in PSUM.

Optimizations over the naive mask-streaming version (2.32ms -> 0.825ms):
- scatter masks generated ON DEVICE per block with one DVE is_equal op
  (dstl column stride-0-broadcast against an iota table) instead of
  streaming 64MB of host masks;
- dma_gather batched in 1024-index calls (HW ucode cap; larger calls
  fault the exec unit) to amortize the ~1us SWDGE descriptor-gen cost
  on the GPSIMD engine — the old 256-index calls made desc-gen the
  global bottleneck (85% Pool engine busy);
- edge scoring uses non-transpose gathers (edges on partitions) with a
  fused scalar_tensor_tensor multiply+accumulate per 128-edge chunk —
  no ones-matmul reduction, scores laid out [128, n/128];
- each table AllGather is split lo/hi (blocks 0-23 / 24-48, both
  sub-tables < 32768 rows for int16 indices): the lo collective fires
  mid-phase so lo-half gathers overlap the producer phase's tail;
- P0 loads x in 4-block batches and all shard writes are batched per
  super-group to cut HWDGE issue count;
- node -> (core, block) assignment is degree-balanced (greedy LPT) to
  shrink the shared max-over-cores aggregation bucket sizes;
- aggregation slots pack per (super-group, half) run with no per-block
  roundup: blocks' masks cover the union of chunks their rows can span
  across cores (straddle rows read dstl=PAD), cutting gather slots from
  124.5k to 109.4k per layer.
Engine occupancy (TimelineSim): DMA 96%, GPSIMD 81%, DVE 62%, PE 34%.
"""
import os
import sys

for p in ("/opt/trn_rl_repo", "/opt/pypackages"):
    if p not in sys.path:
        sys.path.insert(0, p)

import numpy as np

import concourse.bacc as bacc
import concourse.bass as bass
import concourse.mybir as mybir
import concourse.tile as tile
from concourse import bass_utils, library_config
from concourse.masks import make_identity

FP16 = mybir.dt.float16
F32 = mybir.dt.float32
I16 = mybir.dt.int16
AF = mybir.ActivationFunctionType
OP = mybir.AluOpType

NC_CORES = 8
D_IN = 512
D_HID = 256
SUP = 3             # dst blocks per aggregation super-group (one gather pair)
GMAX_NT = 1024      # max idx per non-transpose dma_gather (HW ucode limit)
SC_CALL = 1024      # edges per scoring gather call
PAD_DSTL = 999.0    # dstl pad value; never equals a block-local index


def _wrap_idx(idx, width):
    """int16 gather-index layout: [128, n/16], idx[i] at [i%16, i//16],
    replicated across the 8 groups of 16 partitions."""
    n = len(idx)
    assert n % 16 == 0
    t = np.asarray(idx, np.int16).reshape(n // 16, 16).T  # [16, n/16]
    out = np.tile(t, (8, 1))
    assert out.shape == (128, n // 16)
    if width > n // 16:
        out = np.concatenate(
            [out, np.zeros((128, width - n // 16), np.int16)], axis=1)
    return out


def _wrap_vals(vals, pad, n_slots):
    """Value layout matching non-transpose gather rows: slot i -> [i%128,
    i//128]. Returns [128, n_slots/128] float16."""
    a = np.full(n_slots, pad, np.float32)
    a[:len(vals)] = vals
    return a.reshape(n_slots // 128, 128).T.astype(np.float16)


def prep_host(x, edge_index, W1, b1, W2, b2, n_cores=NC_CORES):
    N, d_in = x.shape
    E = edge_index.shape[1]
    d_hid = W1.shape[1]
    n_per = N // n_cores
    nblk = (n_per + 127) // 128
    n_loc = nblk * 128
    NP = n_loc * n_cores
    # split each core's shard at block LO_BLK: the lo sub-shards AllGather
    # into a [n_cores*lo_loc] table as soon as the first LO_BLK blocks are
    # produced, overlapping the rest of the producing phase. Both tables
    # stay < 32768 rows for int16 gather indices.
    lo_blk = nblk // 2
    lo_loc = lo_blk * 128
    hi_loc = n_loc - lo_loc
    lo_n = lo_loc * n_cores
    hi_n = hi_loc * n_cores
    assert lo_n < 32768 and hi_n < 32768

    src0 = np.asarray(edge_index[0], np.int64)
    dst0 = np.asarray(edge_index[1], np.int64)
    src_f = np.concatenate([src0, np.arange(N, dtype=np.int64)])
    dst_f = np.concatenate([dst0, np.arange(N, dtype=np.int64)])

    deg = np.bincount(dst_f, minlength=N).astype(np.float64)
    dinv = np.where(deg > 0, 1.0 / np.sqrt(deg), 0.0).astype(np.float32)

    # ---- degree-balanced node -> (core, local slot) assignment ----
    # lo membership fixed up front (first lo_n nodes), then greedy LPT
    # packing equalizes each (core, block) bin's (in-deg-from-lo,
    # in-deg-from-hi) so the shared bucket sizes max_c cnt shrink to ~mean.
    n_lo_real = min(lo_n, N)
    node_is_lo = np.zeros(N, bool)
    node_is_lo[:n_lo_real] = True
    src_is_lo = node_is_lo[src_f]
    dl = np.bincount(dst_f[src_is_lo], minlength=N)     # in-deg from lo srcs
    dh = np.bincount(dst_f[~src_is_lo], minlength=N)    # in-deg from hi srcs

    node_core = np.zeros(N, np.int64)
    node_j = np.zeros(N, np.int64)

    def pack(nodes, nbins_per_core, cap_real, j_base):
        nbins = n_cores * nbins_per_core
        s_lo = np.zeros(nbins)
        s_hi = np.zeros(nbins)
        fill = np.zeros(nbins, np.int64)
        core_fill = np.zeros(n_cores, np.int64)
        order_ = np.argsort(-(dl[nodes] + dh[nodes]))
        for v in nodes[order_]:
            score = np.maximum(s_lo + dl[v], s_hi + dh[v]) + \
                (s_lo + dl[v]) + (s_hi + dh[v])
            bad = (fill >= 128) | \
                (core_fill[np.arange(nbins) // nbins_per_core] >= cap_real)
            score[bad] = np.inf
            k = int(np.argmin(score))
            s_lo[k] += dl[v]
            s_hi[k] += dh[v]
            c = k // nbins_per_core
            node_core[v] = c
            node_j[v] = j_base + (k % nbins_per_core) * 128 + fill[k]
            fill[k] += 1
            core_fill[c] += 1

    lo_nodes = np.nonzero(node_is_lo)[0]
    hi_nodes = np.nonzero(~node_is_lo)[0]
    pack(lo_nodes, lo_blk, lo_loc, 0)
    pack(hi_nodes, nblk - lo_blk, n_per - lo_loc, lo_loc)

    # node -> padded slot tables
    node_at = np.full((n_cores, n_loc), -1, np.int64)
    node_at[node_core, node_j] = np.arange(N)

    d_core = node_core[dst_f]
    d_j = node_j[dst_f]
    blk = d_j // 128
    dstl = d_j % 128
    core_of = d_core
    s_core = node_core[src_f]
    s_j = node_j[src_f]
    is_lo = s_j < lo_loc
    # row in the lo/hi sub-table
    rel = np.where(is_lo, s_core * lo_loc + s_j,
                   s_core * hi_loc + (s_j - lo_loc))

    # bucket aggregation edges (incl. self-loops) by (core, block, lo/hi)
    hi_f = (~is_lo).astype(np.int64)
    key = (core_of * nblk + blk) * 2 + hi_f  # bucket id
    order = np.lexsort((rel, key))
    key_s = key[order]
    rel_s = rel[order]
    dstl_s = dstl[order]
    nbuck = n_cores * nblk * 2
    counts = np.bincount(key_s, minlength=nbuck)
    starts = np.concatenate([[0], np.cumsum(counts)])

    def bucket(c, b, h):
        k = (c * nblk + b) * 2 + h
        s, e = starts[k], starts[k + 1]
        return rel_s[s:e], dstl_s[s:e]

    cnt = counts.reshape(n_cores, nblk, 2)

    # super-groups of SUP blocks -> one lo + one hi gather run each.
    # Slots pack per (super, half) with NO per-block padding (the run pads
    # to 128 only at its end, max over cores). Each block's mask covers
    # the union over cores of the chunks its rows can occupy; rows of
    # neighboring blocks inside straddle chunks read dstl=PAD so they
    # never match. Degree balancing keeps the unions tight.
    n_sup = (nblk + SUP - 1) // SUP
    supers = [list(range(g * SUP, min((g + 1) * SUP, nblk)))
              for g in range(n_sup)]
    call_lo, call_hi = [], []
    cb0 = np.zeros((2, nblk), np.int64)  # union chunk range per (half, blk)
    cb1 = np.zeros((2, nblk), np.int64)
    for g, bs in enumerate(supers):
        for h in range(2):
            start = np.zeros((n_cores, len(bs) + 1), np.int64)
            for j, b in enumerate(bs):
                start[:, j + 1] = start[:, j] + cnt[:, b, h]
            run = ((int(start[:, -1].max()) + 127) // 128) * 128
            (call_lo if h == 0 else call_hi).append(run)
            for j, b in enumerate(bs):
                cb0[h, b] = int(start[:, j].min()) // 128
                cb1[h, b] = max((int(start[:, j + 1].max()) + 127) // 128,
                                cb0[h, b] + 1)
    cmax = max(max(call_lo), max(call_hi)) // 128  # chunks per gather tile
    tot_agg = int(sum(call_lo) + sum(call_hi))

    ncl = cb1[0] - cb0[0]                # lo mask chunks per block
    n_ch = ncl + (cb1[1] - cb0[1])       # total mask chunks per block
    nchmax = int(n_ch.max())
    ch_off = np.concatenate([[0], np.cumsum(n_ch)])  # dstl col offsets

    # per-core agg gather indices + per-block dstl columns
    gidx = np.zeros((n_cores, 128, tot_agg // 16), np.int16)
    dstl_t = np.full((n_cores, 128, int(ch_off[-1])), PAD_DSTL, np.float16)
    for c in range(n_cores):
        off = 0
        for g, bs in enumerate(supers):
            for h in range(2):
                run = call_lo[g] if h == 0 else call_hi[g]
                ia = np.zeros(run, np.int64)
                dv = np.full(run, PAD_DSTL, np.float32)
                p = 0
                pos = []
                for b in bs:
                    brel, bdv = bucket(c, b, h)
                    ia[p:p + len(brel)] = brel
                    dv[p:p + len(brel)] = bdv
                    pos.append((p, p + len(brel)))
                    p += len(brel)
                gidx[c, :, off // 16: (off + run) // 16] = \
                    _wrap_idx(ia, run // 16)
                wrapped = dv.reshape(run // 128, 128).T  # [128, chunks]
                for j, b in enumerate(bs):
                    s, e = pos[j]
                    c0, c1 = int(cb0[h, b]), int(cb1[h, b])
                    seg = wrapped[:, c0:c1].astype(np.float32).copy()
                    slot = (np.arange(c0 * 128, c1 * 128)
                            .reshape(c1 - c0, 128).T)
                    seg[(slot < s) | (slot >= e)] = PAD_DSTL
                    co = ch_off[b] + (0 if h == 0 else ncl[b])
                    dstl_t[c, :, co:co + (c1 - c0)] = seg.astype(np.float16)
                off += run
        assert off == tot_agg

    # iota table for mask generation: iota_rep[p, d, c] = d
    iota_rep = np.tile(
        np.arange(128, dtype=np.float16)[None, :, None], (128, 1, nchmax))

    # scoring: original E edges, n_cores contiguous slices, 4-way grouped
    # by (src-half, dst-half). Group 0 (lo,lo) only needs the lo AllGather.
    e_per = E // n_cores
    sc_src = rel[:E]
    sc_dst = np.where(d_j[:E] < lo_loc, d_core[:E] * lo_loc + d_j[:E],
                      d_core[:E] * hi_loc + (d_j[:E] - lo_loc))
    s_hi = (~is_lo[:E]).astype(np.int64)
    d_hi = (d_j[:E] >= lo_loc).astype(np.int64)
    groups = [[None] * 4 for _ in range(n_cores)]
    for c in range(n_cores):
        sl_ = slice(c * e_per, (c + 1) * e_per)
        gg = s_hi[sl_] * 2 + d_hi[sl_]
        for g in range(4):
            groups[c][g] = np.nonzero(gg == g)[0]
    g_sz = [max(len(groups[c][g]) for c in range(n_cores)) for g in range(4)]
    g_sz = [((s + SC_CALL - 1) // SC_CALL) * SC_CALL for s in g_sz]
    tot_sc = sum(g_sz)
    calls = []  # (group, call_size)
    for g in range(4):
        for _ in range(g_sz[g] // SC_CALL):
            calls.append((g, SC_CALL))

    sidx = np.zeros((n_cores, 128, tot_sc // 16), np.int16)
    didx = np.zeros((n_cores, 128, tot_sc // 16), np.int16)
    perm = np.full((n_cores, tot_sc), -1, np.int64)
    for c in range(n_cores):
        off = 0
        for g in range(4):
            idxs = groups[c][g]
            ss = sc_src[c * e_per + idxs]
            dd = sc_dst[c * e_per + idxs]
            sa = np.zeros(g_sz[g], np.int64)
            da = np.zeros(g_sz[g], np.int64)
            sa[:len(idxs)] = ss
            da[:len(idxs)] = dd
            perm[c, off:off + len(idxs)] = idxs
            p = 0
            while p < g_sz[g]:
                s = min(SC_CALL, g_sz[g] - p)
                sidx[c, :, (off + p) // 16:(off + p + s) // 16] = \
                    _wrap_idx(sa[p:p + s], s // 16)
                didx[c, :, (off + p) // 16:(off + p + s) // 16] = \
                    _wrap_idx(da[p:p + s], s // 16)
                p += s
            off += g_sz[g]

    # dense per-core tensors
    W1h = np.asarray(W1, np.float32).reshape(4, 128, d_hid).transpose(1, 0, 2)
    W2h = np.asarray(W2, np.float32).reshape(2, 128, d_hid).transpose(1, 0, 2)
    b1c = np.asarray(b1, np.float32).reshape(2, 128).T.copy()  # [128, 2]
    b2r = np.tile(np.asarray(b2, np.float32)[None, :], (128, 1))  # [128, 256]

    xf = np.asarray(x, np.float32)
    in_maps = []
    for c in range(n_cores):
        sel = node_at[c]  # padded slot -> node id (-1 = empty)
        xs = np.zeros((n_loc, d_in), np.float32)
        xs[sel >= 0] = xf[sel[sel >= 0]]
        xt = np.ascontiguousarray(
            xs.T.reshape(4, 128, nblk, 128).transpose(1, 2, 0, 3))
        dv = np.zeros(n_loc, np.float32)
        dv[sel >= 0] = dinv[sel[sel >= 0]]
        dcol = dv.reshape(nblk, 128).T.copy()  # [128, nblk]
        in_maps.append({
            "xt": xt.astype(np.float16),
            "w1": W1h.astype(np.float16),
            "w2": W2h.astype(np.float16),
            "b1c": b1c, "b2r": b2r,
            "dinv": dcol,
            "gidx": gidx[c],
            "dstl": dstl_t[c],
            "iota": iota_rep,
            "sidx": sidx[c], "didx": didx[c],
        })

    cfg = dict(n_cores=n_cores, N=N, E=E, d_in=d_in, d_hid=d_hid,
               n_per=n_per, nblk=nblk, n_loc=n_loc, NP=NP,
               lo_blk=lo_blk, lo_loc=lo_loc, hi_loc=hi_loc,
               lo_n=lo_n, hi_n=hi_n,
               n_ch=n_ch.tolist(), ncl=ncl.tolist(),
               cb0=cb0.tolist(), cb1=cb1.tolist(),
               nchmax=nchmax, ch_off=ch_off.tolist(),
               supers=supers, call_lo=call_lo, call_hi=call_hi, cmax=cmax,
               tot_agg=tot_agg, g_sz=g_sz, tot_sc=tot_sc, calls=calls,
               e_per=e_per)
    meta = dict(perm=perm, node_at=node_at)
    return in_maps, cfg, meta


def build_nc(cfg, timing_mode=False, debug_stage=0):
    """debug_stage: 0=full, 1=stop after GEMM1 (dump hs1), 2=stop after
    layer-1 agg + GEMM2 (dump hs2), 3=stop after layer-2 agg (dump h2)."""
    n_cores = cfg["n_cores"]
    d_hid = cfg["d_hid"]
    nblk, n_loc, NP = cfg["nblk"], cfg["n_loc"], cfg["NP"]
    lo_blk, lo_loc, hi_loc = cfg["lo_blk"], cfg["lo_loc"], cfg["hi_loc"]
    lo_n, hi_n = cfg["lo_n"], cfg["hi_n"]
    n_ch, ncl = cfg["n_ch"], cfg["ncl"]
    cb0, cb1 = cfg["cb0"], cfg["cb1"]
    nchmax, ch_off = cfg["nchmax"], cfg["ch_off"]
    supers, call_lo, call_hi = cfg["supers"], cfg["call_lo"], cfg["call_hi"]
    cmax = cfg["cmax"]
    tot_agg, tot_sc, calls = cfg["tot_agg"], cfg["tot_sc"], cfg["calls"]

    nc = bacc.Bacc("TRN2", target_bir_lowering=False, debug=False,
                   num_devices=1 if timing_mode else n_cores)

    def all_gather(shard, full):
        if timing_mode:
            # timing workalike: local copy stands in for the collective;
            # real AG cost (6 half-table AGs ~13us each) added by caller
            nc.sync.dma_start(full[0:shard.shape[0], :], shard[:])
        else:
            nc.gpsimd.collective_compute(
                "AllGather", OP.bypass, replica_groups=rg,
                ins=[shard.opt()], outs=[full.opt()])

    t_xt = nc.dram_tensor("xt", [128, nblk, 4, 128], FP16, kind="ExternalInput").ap()
    t_w1 = nc.dram_tensor("w1", [128, 4, d_hid], FP16, kind="ExternalInput").ap()
    t_w2 = nc.dram_tensor("w2", [128, 2, d_hid], FP16, kind="ExternalInput").ap()
    t_b1c = nc.dram_tensor("b1c", [128, 2], F32, kind="ExternalInput").ap()
    t_b2r = nc.dram_tensor("b2r", [128, d_hid], F32, kind="ExternalInput").ap()
    t_dinv = nc.dram_tensor("dinv", [128, nblk], F32, kind="ExternalInput").ap()
    t_gidx = nc.dram_tensor("gidx", [128, tot_agg // 16], I16,
                            kind="ExternalInput").ap()
    t_dstl = nc.dram_tensor("dstl", [128, ch_off[-1]], FP16,
                            kind="ExternalInput").ap()
    t_iota = nc.dram_tensor("iota", [128, 128, nchmax], FP16,
                            kind="ExternalInput").ap()
    t_sidx = nc.dram_tensor("sidx", [128, tot_sc // 16], I16,
                            kind="ExternalInput").ap()
    t_didx = nc.dram_tensor("didx", [128, tot_sc // 16], I16,
                            kind="ExternalInput").ap()
    t_out = nc.dram_tensor("scores", [128, tot_sc // 128], F32,
                           kind="ExternalOutput").ap()
    t_dbg = None
    if debug_stage:
        t_dbg = nc.dram_tensor("dbg", [n_loc, d_hid], FP16,
                               kind="ExternalOutput").ap()

    rg = [list(range(n_cores))]

    with tile.TileContext(nc) as tc:
        with (
            tc.tile_pool(name="const", bufs=1) as cp,
            tc.tile_pool(name="xtp", bufs=3) as xtp,
            tc.tile_pool(name="sbuf", bufs=3) as sb,
            tc.tile_pool(name="gath", bufs=2) as gp,
            tc.tile_pool(name="smask", bufs=2) as smp,
            tc.tile_pool(name="scg", bufs=3) as scp,
            tc.tile_pool(name="scs", bufs=2) as sop,
            tc.tile_pool(name="psum", bufs=3, space="PSUM") as ps,
            tc.tile_pool(name="psum_t", bufs=2, space="PSUM") as pst,
            tc.tile_pool(name="dram", bufs=1, space="DRAM") as dr,
        ):
            nc.gpsimd.load_library(library_config.mlp)

            # ---- resident constants ----
            # only w1 + dinv block the first GEMM; the bulky index tables
            # stream during P0's DMA slack (emitted after the first xt load)
            w1_sb = cp.tile([128, 4, d_hid], FP16)
            nc.sync.dma_start(w1_sb[:], t_w1)
            dinv_sb = cp.tile([128, nblk], F32)
            nc.sync.dma_start(dinv_sb[:], t_dinv)
            w2_sb = cp.tile([128, 2, d_hid], FP16)
            b1c_sb = cp.tile([128, 2], F32)
            b2r_sb = cp.tile([128, d_hid], F32)
            gidx_sb = cp.tile([128, tot_agg // 16], I16)
            dstl_sb = cp.tile([128, ch_off[-1]], FP16)
            iota_sb = cp.tile([128, 128, nchmax], FP16)
            ident = cp.tile([128, 128], FP16)
            h1T = cp.tile([128, nblk, 2, 128], FP16)
            sidx_all = cp.tile([128, tot_sc // 16], I16)
            didx_all = cp.tile([128, tot_sc // 16], I16)

            _regs = {}

            def nreg(n):
                if n not in _regs:
                    _regs[n] = nc.gpsimd.to_reg(n)
                return _regs[n]

            def late_consts():
                nc.sync.dma_start(gidx_sb[:], t_gidx)
                nc.sync.dma_start(dstl_sb[:], t_dstl)
                nc.sync.dma_start(iota_sb[:], t_iota)
                nc.sync.dma_start(w2_sb[:], t_w2)
                nc.sync.dma_start(b1c_sb[:], t_b1c)
                nc.sync.dma_start(b2r_sb[:], t_b2r)
                nc.sync.dma_start(sidx_all[:], t_sidx)
                nc.sync.dma_start(didx_all[:], t_didx)
                make_identity(nc, ident[:])

            # ---- DRAM tables (lo/hi split for phased AllGathers) ----
            hs1_sh_lo = dr.tile([lo_loc, d_hid], FP16)
            hs1_sh_hi = dr.tile([hi_loc, d_hid], FP16)
            hs1_lo = dr.tile([lo_n, d_hid], FP16)
            hs1_hi = dr.tile([hi_n, d_hid], FP16)
            hs2_sh_lo = dr.tile([lo_loc, d_hid], FP16)
            hs2_sh_hi = dr.tile([hi_loc, d_hid], FP16)
            hs2_lo = dr.tile([lo_n, d_hid], FP16)
            hs2_hi = dr.tile([hi_n, d_hid], FP16)
            h2_sh_lo = dr.tile([lo_loc, d_hid], FP16)
            h2_sh_hi = dr.tile([hi_loc, d_hid], FP16)
            h2_lo = dr.tile([lo_n, d_hid], FP16)
            h2_hi = dr.tile([hi_n, d_hid], FP16)

            # ---- P0: GEMM1 + dinv scale -> hs1 shards; AG1a mid-phase ----
            # 4-block batches cut HWDGE issue count 4x (aligned with the
            # lo/hi boundary at lo_blk=24)
            for q in range((nblk + 3) // 4):
                bs4 = list(range(4 * q, min(4 * q + 4, nblk)))
                xt_b = xtp.tile([128, 4, 4, 128], FP16, tag="xtb")
                nc.sync.dma_start(xt_b[:, :len(bs4)],
                                  t_xt[:, bs4[0]:bs4[-1] + 1, :, :])
                if q == 0:
                    late_consts()
                hs4 = sb.tile([128, 4, d_hid], FP16, tag="hs4")
                for j, b in enumerate(bs4):
                    g1 = ps.tile([128, d_hid], F32, tag="mm")
                    for k in range(4):
                        nc.tensor.matmul(
                            g1[:], lhsT=xt_b[:, j, k, :],
                            rhs=w1_sb[:, k, :], start=(k == 0), stop=(k == 3))
                    nc.vector.tensor_scalar(hs4[:, j, :], g1[:],
                                            dinv_sb[:, b:b + 1], None, OP.mult)
                b0, b1_ = bs4[0], bs4[-1] + 1
                if b1_ <= lo_blk:
                    dst = hs1_sh_lo[128 * b0:128 * b1_, :]
                else:
                    dst = hs1_sh_hi[128 * (b0 - lo_blk):128 * (b1_ - lo_blk), :]
                nc.sync.dma_start(
                    dst.rearrange("(j p) f -> p j f", p=128), hs4[:, :len(bs4)])
                if b1_ == lo_blk:
                    all_gather(hs1_sh_lo, hs1_lo)
            all_gather(hs1_sh_hi, hs1_hi)

            # ---- aggregation layer: per super-group gather, per block mask
            #      gen + scatter matmuls; post(b, o_psum) consumes output ----
            def agg_layer(tab_lo, tab_hi, post, sh_out=None, hook=None):
                # post(b, o, wslot) fills wslot with the block's [128, d_hid]
                # fp16 result; results batch-write per super into sh_out
                sh_lo_t, sh_hi_t = sh_out if sh_out else (None, None)
                off = 0
                offs = []
                for g in range(len(supers)):
                    offs.append((off, off + call_lo[g]))
                    off += call_lo[g] + call_hi[g]
                for g, bs in enumerate(supers):
                    lo_off, hi_off = offs[g]
                    g_lo = gp.tile([128, cmax, d_hid], FP16, tag="g_lo")
                    p = 0
                    while p < call_lo[g]:
                        q = min(GMAX_NT, call_lo[g] - p)
                        nc.gpsimd.dma_gather(
                            g_lo[:, p // 128:(p + q) // 128, :],
                            tab_lo[:],
                            gidx_sb[:, (lo_off + p) // 16:(lo_off + p + q) // 16],
                            q, nreg(q), d_hid)
                        p += q
                    g_hi = gp.tile([128, cmax, d_hid], FP16, tag="g_hi")
                    p = 0
                    while p < call_hi[g]:
                        q = min(GMAX_NT, call_hi[g] - p)
                        nc.gpsimd.dma_gather(
                            g_hi[:, p // 128:(p + q) // 128, :],
                            tab_hi[:],
                            gidx_sb[:, (hi_off + p) // 16:(hi_off + p + q) // 16],
                            q, nreg(q), d_hid)
                        p += q
                    wtile = sb.tile([128, SUP, d_hid], FP16, tag="wsup")
                    for j, b in enumerate(bs):
                        ncb = n_ch[b]
                        s_b = smp.tile([128, 128, nchmax], FP16, tag="s")
                        d_bc = dstl_sb[:, ch_off[b]:ch_off[b] + ncb] \
                            .unsqueeze(1).broadcast_to([128, 128, ncb])
                        nc.vector.tensor_tensor(
                            s_b[:, :, :ncb], d_bc, iota_sb[:, :, :ncb],
                            OP.is_equal)
                        o = ps.tile([128, d_hid], F32, tag="mm")
                        nlo = ncl[b]
                        for c in range(ncb):
                            if c < nlo:
                                rhs = g_lo[:, cb0[0][b] + c, :]
                            else:
                                rhs = g_hi[:, cb0[1][b] + (c - nlo), :]
                            nc.tensor.matmul(o[:], lhsT=s_b[:, :, c], rhs=rhs,
                                             start=(c == 0), stop=(c == ncb - 1))
                        post(b, o, wtile[:, j, :])
                    if sh_out:
                        b0, b1_ = bs[0], bs[-1] + 1
                        if b1_ <= lo_blk:
                            dst = sh_lo_t[128 * b0:128 * b1_, :]
                        else:
                            dst = sh_hi_t[128 * (b0 - lo_blk):
                                          128 * (b1_ - lo_blk), :]
                        nc.sync.dma_start(
                            dst.rearrange("(j p) f -> p j f", p=128),
                            wtile[:, :len(bs), :])
                    if hook is not None:
                        hook(g)

            # ---- P2+P3 fused: layer-1 aggregation -> h1T; GEMM2 -> hs2 ----
            def post1(b, o, wslot):
                tmp = sb.tile([128, d_hid], FP16, tag="tmp")
                nc.vector.tensor_scalar(tmp[:], o[:], dinv_sb[:, b:b + 1],
                                        None, OP.mult)
                for h in range(2):
                    tp = pst.tile([128, 128], FP16, tag="tps")
                    nc.tensor.transpose(tp[:], tmp[:, 128 * h:128 * (h + 1)],
                                        ident[:])
                    nc.scalar.activation(h1T[:, b, h, :], tp[:], AF.Relu,
                                         bias=b1c_sb[:, h:h + 1])
                g2 = ps.tile([128, d_hid], F32, tag="mm")
                for k in range(2):
                    nc.tensor.matmul(g2[:], lhsT=h1T[:, b, k, :],
                                     rhs=w2_sb[:, k, :],
                                     start=(k == 0), stop=(k == 1))
                nc.vector.tensor_scalar(wslot, g2[:], dinv_sb[:, b:b + 1],
                                        None, OP.mult)

            # ---- P5: layer-2 aggregation -> h2 shards ----
            def post2(b, o, wslot):
                nc.vector.scalar_tensor_tensor(
                    wslot, o[:], dinv_sb[:, b:b + 1], b2r_sb[:],
                    OP.mult, OP.add)

            # emit the lo-half AllGather as soon as blocks 0..lo_blk-1 are
            # written (mid-aggregation), overlapping it with the hi half
            ag_super = (lo_blk - 1) // SUP  # super index that completes lo

            def dump(sh_lo, sh_hi):
                nc.sync.dma_start(t_dbg[0:lo_loc, :], sh_lo[:])
                nc.sync.dma_start(t_dbg[lo_loc:n_loc, :], sh_hi[:])

            stop = False
            if debug_stage == 1:
                dump(hs1_sh_lo, hs1_sh_hi)
                stop = True
            if not stop:
                agg_layer(hs1_lo, hs1_hi, post1,
                          sh_out=(hs2_sh_lo, hs2_sh_hi),
                          hook=lambda g: all_gather(hs2_sh_lo, hs2_lo)
                          if g == ag_super else None)
                if debug_stage == 2:
                    dump(hs2_sh_lo, hs2_sh_hi)
                    stop = True
            if not stop:
                all_gather(hs2_sh_hi, hs2_hi)
                agg_layer(hs2_lo, hs2_hi, post2,
                          sh_out=(h2_sh_lo, h2_sh_hi),
                          hook=lambda g: all_gather(h2_sh_lo, h2_lo)
                          if g == ag_super else None)
                if debug_stage == 3:
                    dump(h2_sh_lo, h2_sh_hi)
                    stop = True
            if not stop:
                all_gather(h2_sh_hi, h2_hi)

            # ---- P7: edge scoring (edges-on-partitions; fused mult+accum) ----
            off = 0
            for (grp, csz) in (calls if not stop else []):
                s_tab = h2_hi if grp >= 2 else h2_lo
                d_tab = h2_hi if grp % 2 == 1 else h2_lo
                nch_sc = csz // 128
                gt = scp.tile([128, nch_sc, d_hid], FP16, tag="sc_g")
                nc.gpsimd.dma_gather(
                    gt[:], s_tab[:],
                    sidx_all[:, off // 16:(off + csz) // 16], csz, nreg(csz), d_hid)
                dt_ = scp.tile([128, nch_sc, d_hid], FP16, tag="sc_d")
                nc.gpsimd.dma_gather(
                    dt_[:], d_tab[:],
                    didx_all[:, off // 16:(off + csz) // 16], csz, nreg(csz), d_hid)
                sc_acc = sop.tile([128, nch_sc], F32, tag="scacc")
                for w in range(nch_sc):
                    junk = sb.tile([128, d_hid], FP16, tag="junk")
                    nc.vector.scalar_tensor_tensor(
                        junk[:], gt[:, w, :], 1.0, dt_[:, w, :],
                        OP.mult, OP.mult, accum_out=sc_acc[:, w:w + 1])
                sc_sb = sop.tile([128, nch_sc], F32, tag="scsb")
                nc.scalar.activation(sc_sb[:], sc_acc[:], AF.Sigmoid)
                nc.sync.dma_start(
                    t_out[:, off // 128:(off + csz) // 128], sc_sb[:])
                off += csz

    nc.compile()
    return nc


def _run(in_maps, cfg, meta, trace=False):
    nc = build_nc(cfg)
    res = bass_utils.run_bass_kernel_spmd(
        nc, in_maps, core_ids=list(range(cfg["n_cores"])), trace=trace)
    perm = meta["perm"]
    E, e_per = cfg["E"], cfg["e_per"]
    out = np.zeros(E, np.float32)
    for c in range(cfg["n_cores"]):
        # scores[p, ch] holds slot ch*128+p
        sc = np.asarray(res.results[c]["scores"], np.float32).T.reshape(-1)
        valid = perm[c] >= 0
        out[c * e_per + perm[c][valid]] = sc[valid]
    return out, res


def kernel(x, edge_index, W1, b1, W2, b2):
    in_maps, cfg, meta = prep_host(
        np.asarray(x), np.asarray(edge_index), np.asarray(W1),
        np.asarray(b1), np.asarray(W2), np.asarray(b2))
    out, _res = _run(in_maps, cfg, meta,
                     trace=bool(int(os.environ.get("KERNEL_TRACE", "0"))))
    return out


# revision 65
# speedup vs baseline: 1.0033x; 1.0033x over previous
"""Two-layer GCN + edge scoring on 8 Trainium2 NeuronCores.

Sharding: nodes row-sharded across cores (6250/core, padded to 6272 = 49
blocks of 128); aggregation edges partitioned by destination core and
grouped by destination block; weights replicated; three device-side
AllGathers move the per-node tables (hs1, hs2, h2) between phases.

Math trick: the GCN symmetric norm dinv[src]*dinv[dst] is separable, so
the gathered tables are pre-scaled by dinv (hs = dinv * (x@W)) and block
outputs post-scaled by dinv — the segment-sum masks stay pure 0/1 and the
scatter-add becomes S^T @ gathered_rows matmuls accumulated in PSUM.

Optimizations over the naive mask-streaming version (2.32ms -> 0.825ms):
- scatter masks generated ON DEVICE per block with one DVE is_equal op
  (dstl column stride-0-broadcast against an iota table) instead of
  streaming 64MB of host masks;
- dma_gather batched in 1024-index calls (HW ucode cap; larger calls
  fault the exec unit) to amortize the ~1us SWDGE descriptor-gen cost
  on the GPSIMD engine — the old 256-index calls made desc-gen the
  global bottleneck (85% Pool engine busy);
- edge scoring uses non-transpose gathers (edges on partitions) with a
  fused scalar_tensor_tensor multiply+accumulate per 128-edge chunk —
  no ones-matmul reduction, scores laid out [128, n/128];
- each table AllGather is split lo/hi (blocks 0-23 / 24-48, both
  sub-tables < 32768 rows for int16 indices): the lo collective fires
  mid-phase so lo-half gathers overlap the producer phase's tail;
- P0 loads x in 4-block batches and all shard writes are batched per
  super-group to cut HWDGE issue count;
- node -> (core, block) assignment is degree-balanced (greedy LPT) to
  shrink the shared max-over-cores aggregation bucket sizes;
- aggregation slots pack per (super-group, half) run with no per-block
  roundup: blocks' masks cover the union of chunks their rows can span
  across cores (straddle rows read dstl=PAD), cutting gather slots from
  124.5k to 109.4k per layer.
Engine occupancy (TimelineSim): DMA 96%, GPSIMD 81%, DVE 62%, PE 34%.
"""
import os
import sys

for p in ("/opt/trn_rl_repo", "/opt/pypackages"):
    if p not in sys.path:
        sys.path.insert(0, p)

import numpy as np

import concourse.bacc as bacc
import concourse.bass as bass
import concourse.mybir as mybir
import concourse.tile as tile
from concourse import bass_utils, library_config
from concourse.masks import make_identity

FP16 = mybir.dt.float16
F32 = mybir.dt.float32
I16 = mybir.dt.int16
AF = mybir.ActivationFunctionType
OP = mybir.AluOpType

NC_CORES = 8
D_IN = 512
D_HID = 256
SUP = 3             # dst blocks per aggregation super-group (one gather pair)
GMAX_NT = 1024      # max idx per non-transpose dma_gather (HW ucode limit)
SC_CALL = 1024      # edges per scoring gather call
PAD_DSTL = 999.0    # dstl pad value; never equals a block-local index


def _wrap_idx(idx, width):
    """int16 gather-index layout: [128, n/16], idx[i] at [i%16, i//16],
    replicated across the 8 groups of 16 partitions."""
    n = len(idx)
    assert n % 16 == 0
    t = np.asarray(idx, np.int16).reshape(n // 16, 16).T  # [16, n/16]
    out = np.tile(t, (8, 1))
    assert out.shape == (128, n // 16)
    if width > n // 16:
        out = np.concatenate(
            [out, np.zeros((128, width - n // 16), np.int16)], axis=1)
    return out


def _wrap_vals(vals, pad, n_slots):
    """Value layout matching non-transpose gather rows: slot i -> [i%128,
    i//128]. Returns [128, n_slots/128] float16."""
    a = np.full(n_slots, pad, np.float32)
    a[:len(vals)] = vals
    return a.reshape(n_slots // 128, 128).T.astype(np.float16)


def prep_host(x, edge_index, W1, b1, W2, b2, n_cores=NC_CORES):
    N, d_in = x.shape
    E = edge_index.shape[1]
    d_hid = W1.shape[1]
    n_per = N // n_cores
    nblk = (n_per + 127) // 128
    n_loc = nblk * 128
    NP = n_loc * n_cores
    # split each core's shard at block LO_BLK: the lo sub-shards AllGather
    # into a [n_cores*lo_loc] table as soon as the first LO_BLK blocks are
    # produced, overlapping the rest of the producing phase. Both tables
    # stay < 32768 rows for int16 gather indices.
    lo_blk = nblk // 2
    lo_loc = lo_blk * 128
    hi_loc = n_loc - lo_loc
    lo_n = lo_loc * n_cores
    hi_n = hi_loc * n_cores
    assert lo_n < 32768 and hi_n < 32768

    src0 = np.asarray(edge_index[0], np.int64)
    dst0 = np.asarray(edge_index[1], np.int64)
    src_f = np.concatenate([src0, np.arange(N, dtype=np.int64)])
    dst_f = np.concatenate([dst0, np.arange(N, dtype=np.int64)])

    deg = np.bincount(dst_f, minlength=N).astype(np.float64)
    dinv = np.where(deg > 0, 1.0 / np.sqrt(deg), 0.0).astype(np.float32)

    # ---- degree-balanced node -> (core, local slot) assignment ----
    # lo membership fixed up front (first lo_n nodes), then greedy LPT
    # packing equalizes each (core, block) bin's (in-deg-from-lo,
    # in-deg-from-hi) so the shared bucket sizes max_c cnt shrink to ~mean.
    n_lo_real = min(lo_n, N)
    node_is_lo = np.zeros(N, bool)
    node_is_lo[:n_lo_real] = True
    src_is_lo = node_is_lo[src_f]
    dl = np.bincount(dst_f[src_is_lo], minlength=N)     # in-deg from lo srcs
    dh = np.bincount(dst_f[~src_is_lo], minlength=N)    # in-deg from hi srcs

    node_core = np.zeros(N, np.int64)
    node_j = np.zeros(N, np.int64)

    def pack(nodes, nbins_per_core, cap_real, j_base):
        nbins = n_cores * nbins_per_core
        s_lo = np.zeros(nbins)
        s_hi = np.zeros(nbins)
        fill = np.zeros(nbins, np.int64)
        core_fill = np.zeros(n_cores, np.int64)
        order_ = np.argsort(-(dl[nodes] + dh[nodes]))
        for v in nodes[order_]:
            score = np.maximum(s_lo + dl[v], s_hi + dh[v]) + \
                (s_lo + dl[v]) + (s_hi + dh[v])
            bad = (fill >= 128) | \
                (core_fill[np.arange(nbins) // nbins_per_core] >= cap_real)
            score[bad] = np.inf
            k = int(np.argmin(score))
            s_lo[k] += dl[v]
            s_hi[k] += dh[v]
            c = k // nbins_per_core
            node_core[v] = c
            node_j[v] = j_base + (k % nbins_per_core) * 128 + fill[k]
            fill[k] += 1
            core_fill[c] += 1

    lo_nodes = np.nonzero(node_is_lo)[0]
    hi_nodes = np.nonzero(~node_is_lo)[0]
    pack(lo_nodes, lo_blk, lo_loc, 0)
    pack(hi_nodes, nblk - lo_blk, n_per - lo_loc, lo_loc)

    # node -> padded slot tables
    node_at = np.full((n_cores, n_loc), -1, np.int64)
    node_at[node_core, node_j] = np.arange(N)

    d_core = node_core[dst_f]
    d_j = node_j[dst_f]
    blk = d_j // 128
    dstl = d_j % 128
    core_of = d_core
    s_core = node_core[src_f]
    s_j = node_j[src_f]
    is_lo = s_j < lo_loc
    # row in the lo/hi sub-table
    rel = np.where(is_lo, s_core * lo_loc + s_j,
                   s_core * hi_loc + (s_j - lo_loc))

    # bucket aggregation edges (incl. self-loops) by (core, block, lo/hi)
    hi_f = (~is_lo).astype(np.int64)
    key = (core_of * nblk + blk) * 2 + hi_f  # bucket id
    order = np.lexsort((rel, key))
    key_s = key[order]
    rel_s = rel[order]
    dstl_s = dstl[order]
    nbuck = n_cores * nblk * 2
    counts = np.bincount(key_s, minlength=nbuck)
    starts = np.concatenate([[0], np.cumsum(counts)])

    def bucket(c, b, h):
        k = (c * nblk + b) * 2 + h
        s, e = starts[k], starts[k + 1]
        return rel_s[s:e], dstl_s[s:e]

    cnt = counts.reshape(n_cores, nblk, 2)

    # super-groups of SUP blocks -> one lo + one hi gather run each.
    # Slots pack per (super, half) with NO per-block padding (the run pads
    # to 128 only at its end, max over cores). Each block's mask covers
    # the union over cores of the chunks its rows can occupy; rows of
    # neighboring blocks inside straddle chunks read dstl=PAD so they
    # never match. Degree balancing keeps the unions tight.
    n_sup = (nblk + SUP - 1) // SUP
    supers = [list(range(g * SUP, min((g + 1) * SUP, nblk)))
              for g in range(n_sup)]
    call_lo, call_hi = [], []
    cb0 = np.zeros((2, nblk), np.int64)  # union chunk range per (half, blk)
    cb1 = np.zeros((2, nblk), np.int64)
    for g, bs in enumerate(supers):
        for h in range(2):
            start = np.zeros((n_cores, len(bs) + 1), np.int64)
            for j, b in enumerate(bs):
                start[:, j + 1] = start[:, j] + cnt[:, b, h]
            run = ((int(start[:, -1].max()) + 127) // 128) * 128
            (call_lo if h == 0 else call_hi).append(run)
            for j, b in enumerate(bs):
                cb0[h, b] = int(start[:, j].min()) // 128
                cb1[h, b] = max((int(start[:, j + 1].max()) + 127) // 128,
                                cb0[h, b] + 1)
    cmax = max(max(call_lo), max(call_hi)) // 128  # chunks per gather tile
    tot_agg = int(sum(call_lo) + sum(call_hi))

    ncl = cb1[0] - cb0[0]                # lo mask chunks per block
    n_ch = ncl + (cb1[1] - cb0[1])       # total mask chunks per block
    nchmax = int(n_ch.max())
    ch_off = np.concatenate([[0], np.cumsum(n_ch)])  # dstl col offsets

    # per-core agg gather indices + per-block dstl columns
    gidx = np.zeros((n_cores, 128, tot_agg // 16), np.int16)
    dstl_t = np.full((n_cores, 128, int(ch_off[-1])), PAD_DSTL, np.float16)
    for c in range(n_cores):
        off = 0
        for g, bs in enumerate(supers):
            for h in range(2):
                run = call_lo[g] if h == 0 else call_hi[g]
                ia = np.zeros(run, np.int64)
                dv = np.full(run, PAD_DSTL, np.float32)
                p = 0
                pos = []
                for b in bs:
                    brel, bdv = bucket(c, b, h)
                    ia[p:p + len(brel)] = brel
                    dv[p:p + len(brel)] = bdv
                    pos.append((p, p + len(brel)))
                    p += len(brel)
                gidx[c, :, off // 16: (off + run) // 16] = \
                    _wrap_idx(ia, run // 16)
                wrapped = dv.reshape(run // 128, 128).T  # [128, chunks]
                for j, b in enumerate(bs):
                    s, e = pos[j]
                    c0, c1 = int(cb0[h, b]), int(cb1[h, b])
                    seg = wrapped[:, c0:c1].astype(np.float32).copy()
                    slot = (np.arange(c0 * 128, c1 * 128)
                            .reshape(c1 - c0, 128).T)
                    seg[(slot < s) | (slot >= e)] = PAD_DSTL
                    co = ch_off[b] + (0 if h == 0 else ncl[b])
                    dstl_t[c, :, co:co + (c1 - c0)] = seg.astype(np.float16)
                off += run
        assert off == tot_agg

    # iota table for mask generation: iota_rep[p, d, c] = d
    iota_rep = np.tile(
        np.arange(128, dtype=np.float16)[None, :, None], (128, 1, nchmax))

    # scoring: original E edges, n_cores contiguous slices, 4-way grouped
    # by (src-half, dst-half). Group 0 (lo,lo) only needs the lo AllGather.
    e_per = E // n_cores
    sc_src = rel[:E]
    sc_dst = np.where(d_j[:E] < lo_loc, d_core[:E] * lo_loc + d_j[:E],
                      d_core[:E] * hi_loc + (d_j[:E] - lo_loc))
    s_hi = (~is_lo[:E]).astype(np.int64)
    d_hi = (d_j[:E] >= lo_loc).astype(np.int64)
    groups = [[None] * 4 for _ in range(n_cores)]
    for c in range(n_cores):
        sl_ = slice(c * e_per, (c + 1) * e_per)
        gg = s_hi[sl_] * 2 + d_hi[sl_]
        for g in range(4):
            groups[c][g] = np.nonzero(gg == g)[0]
    g_sz = [max(len(groups[c][g]) for c in range(n_cores)) for g in range(4)]
    g_sz = [((s + SC_CALL - 1) // SC_CALL) * SC_CALL for s in g_sz]
    tot_sc = sum(g_sz)
    calls = []  # (group, call_size)
    for g in range(4):
        for _ in range(g_sz[g] // SC_CALL):
            calls.append((g, SC_CALL))

    sidx = np.zeros((n_cores, 128, tot_sc // 16), np.int16)
    didx = np.zeros((n_cores, 128, tot_sc // 16), np.int16)
    perm = np.full((n_cores, tot_sc), -1, np.int64)
    for c in range(n_cores):
        off = 0
        for g in range(4):
            idxs = groups[c][g]
            ss = sc_src[c * e_per + idxs]
            dd = sc_dst[c * e_per + idxs]
            sa = np.zeros(g_sz[g], np.int64)
            da = np.zeros(g_sz[g], np.int64)
            sa[:len(idxs)] = ss
            da[:len(idxs)] = dd
            perm[c, off:off + len(idxs)] = idxs
            p = 0
            while p < g_sz[g]:
                s = min(SC_CALL, g_sz[g] - p)
                sidx[c, :, (off + p) // 16:(off + p + s) // 16] = \
                    _wrap_idx(sa[p:p + s], s // 16)
                didx[c, :, (off + p) // 16:(off + p + s) // 16] = \
                    _wrap_idx(da[p:p + s], s // 16)
                p += s
            off += g_sz[g]

    # dense per-core tensors
    W1h = np.asarray(W1, np.float32).reshape(4, 128, d_hid).transpose(1, 0, 2)
    W2h = np.asarray(W2, np.float32).reshape(2, 128, d_hid).transpose(1, 0, 2)
    b1c = np.asarray(b1, np.float32).reshape(2, 128).T.copy()  # [128, 2]
    b2r = np.tile(np.asarray(b2, np.float32)[None, :], (128, 1))  # [128, 256]

    xf = np.asarray(x, np.float32)
    in_maps = []
    for c in range(n_cores):
        sel = node_at[c]  # padded slot -> node id (-1 = empty)
        xs = np.zeros((n_loc, d_in), np.float32)
        xs[sel >= 0] = xf[sel[sel >= 0]]
        xt = np.ascontiguousarray(
            xs.T.reshape(4, 128, nblk, 128).transpose(1, 2, 0, 3))
        dv = np.zeros(n_loc, np.float32)
        dv[sel >= 0] = dinv[sel[sel >= 0]]
        dcol = dv.reshape(nblk, 128).T.copy()  # [128, nblk]
        in_maps.append({
            "xt": xt.astype(np.float16),
            "w1": W1h.astype(np.float16),
            "w2": W2h.astype(np.float16),
            "b1c": b1c, "b2r": b2r,
            "dinv": dcol,
            "gidx": gidx[c],
            "dstl": dstl_t[c],
            "iota": iota_rep,
            "sidx": sidx[c], "didx": didx[c],
        })

    cfg = dict(n_cores=n_cores, N=N, E=E, d_in=d_in, d_hid=d_hid,
               n_per=n_per, nblk=nblk, n_loc=n_loc, NP=NP,
               lo_blk=lo_blk, lo_loc=lo_loc, hi_loc=hi_loc,
               lo_n=lo_n, hi_n=hi_n,
               n_ch=n_ch.tolist(), ncl=ncl.tolist(),
               cb0=cb0.tolist(), cb1=cb1.tolist(),
               nchmax=nchmax, ch_off=ch_off.tolist(),
               supers=supers, call_lo=call_lo, call_hi=call_hi, cmax=cmax,
               tot_agg=tot_agg, g_sz=g_sz, tot_sc=tot_sc, calls=calls,
               e_per=e_per)
    meta = dict(perm=perm, node_at=node_at)
    return in_maps, cfg, meta


def build_nc(cfg, timing_mode=False, debug_stage=0):
    """debug_stage: 0=full, 1=stop after GEMM1 (dump hs1), 2=stop after
    layer-1 agg + GEMM2 (dump hs2), 3=stop after layer-2 agg (dump h2)."""
    n_cores = cfg["n_cores"]
    d_hid = cfg["d_hid"]
    nblk, n_loc, NP = cfg["nblk"], cfg["n_loc"], cfg["NP"]
    lo_blk, lo_loc, hi_loc = cfg["lo_blk"], cfg["lo_loc"], cfg["hi_loc"]
    lo_n, hi_n = cfg["lo_n"], cfg["hi_n"]
    n_ch, ncl = cfg["n_ch"], cfg["ncl"]
    cb0, cb1 = cfg["cb0"], cfg["cb1"]
    nchmax, ch_off = cfg["nchmax"], cfg["ch_off"]
    supers, call_lo, call_hi = cfg["supers"], cfg["call_lo"], cfg["call_hi"]
    cmax = cfg["cmax"]
    tot_agg, tot_sc, calls = cfg["tot_agg"], cfg["tot_sc"], cfg["calls"]

    nc = bacc.Bacc("TRN2", target_bir_lowering=False, debug=False,
                   num_devices=1 if timing_mode else n_cores)

    def all_gather(shard, full):
        if timing_mode:
            # timing workalike: local copy stands in for the collective;
            # real AG cost (6 half-table AGs ~13us each) added by caller
            nc.sync.dma_start(full[0:shard.shape[0], :], shard[:])
        else:
            nc.gpsimd.collective_compute(
                "AllGather", OP.bypass, replica_groups=rg,
                ins=[shard.opt()], outs=[full.opt()])

    t_xt = nc.dram_tensor("xt", [128, nblk, 4, 128], FP16, kind="ExternalInput").ap()
    t_w1 = nc.dram_tensor("w1", [128, 4, d_hid], FP16, kind="ExternalInput").ap()
    t_w2 = nc.dram_tensor("w2", [128, 2, d_hid], FP16, kind="ExternalInput").ap()
    t_b1c = nc.dram_tensor("b1c", [128, 2], F32, kind="ExternalInput").ap()
    t_b2r = nc.dram_tensor("b2r", [128, d_hid], F32, kind="ExternalInput").ap()
    t_dinv = nc.dram_tensor("dinv", [128, nblk], F32, kind="ExternalInput").ap()
    t_gidx = nc.dram_tensor("gidx", [128, tot_agg // 16], I16,
                            kind="ExternalInput").ap()
    t_dstl = nc.dram_tensor("dstl", [128, ch_off[-1]], FP16,
                            kind="ExternalInput").ap()
    t_iota = nc.dram_tensor("iota", [128, 128, nchmax], FP16,
                            kind="ExternalInput").ap()
    t_sidx = nc.dram_tensor("sidx", [128, tot_sc // 16], I16,
                            kind="ExternalInput").ap()
    t_didx = nc.dram_tensor("didx", [128, tot_sc // 16], I16,
                            kind="ExternalInput").ap()
    t_out = nc.dram_tensor("scores", [128, tot_sc // 128], F32,
                           kind="ExternalOutput").ap()
    t_dbg = None
    if debug_stage:
        t_dbg = nc.dram_tensor("dbg", [n_loc, d_hid], FP16,
                               kind="ExternalOutput").ap()

    rg = [list(range(n_cores))]

    with tile.TileContext(nc) as tc:
        with (
            tc.tile_pool(name="const", bufs=1) as cp,
            tc.tile_pool(name="xtp", bufs=3) as xtp,
            tc.tile_pool(name="sbuf", bufs=3) as sb,
            tc.tile_pool(name="gath", bufs=2) as gp,
            tc.tile_pool(name="smask", bufs=2) as smp,
            tc.tile_pool(name="scg", bufs=3) as scp,
            tc.tile_pool(name="scs", bufs=2) as sop,
            tc.tile_pool(name="psum", bufs=3, space="PSUM") as ps,
            tc.tile_pool(name="psum_t", bufs=2, space="PSUM") as pst,
            tc.tile_pool(name="dram", bufs=1, space="DRAM") as dr,
        ):
            nc.gpsimd.load_library(library_config.mlp)

            # ---- resident constants ----
            # only w1 + dinv block the first GEMM; the bulky index tables
            # stream during P0's DMA slack (emitted after the first xt load)
            w1_sb = cp.tile([128, 4, d_hid], FP16)
            nc.sync.dma_start(w1_sb[:], t_w1)
            dinv_sb = cp.tile([128, nblk], F32)
            nc.sync.dma_start(dinv_sb[:], t_dinv)
            w2_sb = cp.tile([128, 2, d_hid], FP16)
            b1c_sb = cp.tile([128, 2], F32)
            b2r_sb = cp.tile([128, d_hid], F32)
            gidx_sb = cp.tile([128, tot_agg // 16], I16)
            dstl_sb = cp.tile([128, ch_off[-1]], FP16)
            iota_sb = cp.tile([128, 128, nchmax], FP16)
            ident = cp.tile([128, 128], FP16)
            h1T = cp.tile([128, nblk, 2, 128], FP16)
            sidx_all = cp.tile([128, tot_sc // 16], I16)
            didx_all = cp.tile([128, tot_sc // 16], I16)

            _regs = {}

            def nreg(n):
                if n not in _regs:
                    _regs[n] = nc.gpsimd.to_reg(n)
                return _regs[n]

            def late_consts():
                nc.sync.dma_start(gidx_sb[:], t_gidx)
                nc.sync.dma_start(dstl_sb[:], t_dstl)
                nc.sync.dma_start(iota_sb[:], t_iota)
                nc.sync.dma_start(w2_sb[:], t_w2)
                nc.sync.dma_start(b1c_sb[:], t_b1c)
                nc.sync.dma_start(b2r_sb[:], t_b2r)
                nc.sync.dma_start(sidx_all[:], t_sidx)
                nc.sync.dma_start(didx_all[:], t_didx)
                make_identity(nc, ident[:])

            # ---- DRAM tables (lo/hi split for phased AllGathers) ----
            hs1_sh_lo = dr.tile([lo_loc, d_hid], FP16)
            hs1_sh_hi = dr.tile([hi_loc, d_hid], FP16)
            hs1_lo = dr.tile([lo_n, d_hid], FP16)
            hs1_hi = dr.tile([hi_n, d_hid], FP16)
            hs2_sh_lo = dr.tile([lo_loc, d_hid], FP16)
            hs2_sh_hi = dr.tile([hi_loc, d_hid], FP16)
            hs2_lo = dr.tile([lo_n, d_hid], FP16)
            hs2_hi = dr.tile([hi_n, d_hid], FP16)
            h2_sh_lo = dr.tile([lo_loc, d_hid], FP16)
            h2_sh_hi = dr.tile([hi_loc, d_hid], FP16)
            h2_lo = dr.tile([lo_n, d_hid], FP16)
            h2_hi = dr.tile([hi_n, d_hid], FP16)

            # ---- P0: GEMM1 + dinv scale -> hs1 shards; AG1a mid-phase ----
            # 4-block batches cut HWDGE issue count 4x (aligned with the
            # lo/hi boundary at lo_blk=24)
            for q in range((nblk + 3) // 4):
                bs4 = list(range(4 * q, min(4 * q + 4, nblk)))
                xt_b = xtp.tile([128, 4, 4, 128], FP16, tag="xtb")
                nc.sync.dma_start(xt_b[:, :len(bs4)],
                                  t_xt[:, bs4[0]:bs4[-1] + 1, :, :])
                if q == 0:
                    late_consts()
                hs4 = sb.tile([128, 4, d_hid], FP16, tag="hs4")
                for j, b in enumerate(bs4):
                    g1 = ps.tile([128, d_hid], F32, tag="mm")
                    for k in range(4):
                        nc.tensor.matmul(
                            g1[:], lhsT=xt_b[:, j, k, :],
                            rhs=w1_sb[:, k, :], start=(k == 0), stop=(k == 3))
                    nc.vector.tensor_scalar(hs4[:, j, :], g1[:],
                                            dinv_sb[:, b:b + 1], None, OP.mult)
                b0, b1_ = bs4[0], bs4[-1] + 1
                if b1_ <= lo_blk:
                    dst = hs1_sh_lo[128 * b0:128 * b1_, :]
                else:
                    dst = hs1_sh_hi[128 * (b0 - lo_blk):128 * (b1_ - lo_blk), :]
                nc.sync.dma_start(
                    dst.rearrange("(j p) f -> p j f", p=128), hs4[:, :len(bs4)])
                if b1_ == lo_blk:
                    all_gather(hs1_sh_lo, hs1_lo)
            all_gather(hs1_sh_hi, hs1_hi)

            # ---- aggregation layer: per super-group gather, per block mask
            #      gen + scatter matmuls; post(b, o_psum) consumes output ----
            def agg_layer(tab_lo, tab_hi, post, sh_out=None, hook=None,
                          borrow=False):
                # post(b, o, wslot) fills wslot with the block's [128, d_hid]
                # fp16 result; results batch-write per super into sh_out
                sh_lo_t, sh_hi_t = sh_out if sh_out else (None, None)
                off = 0
                offs = []
                for g in range(len(supers)):
                    offs.append((off, off + call_lo[g]))
                    off += call_lo[g] + call_hi[g]
                for g, bs in enumerate(supers):
                    lo_off, hi_off = offs[g]
                    # first super of layer 2: gather into idle scoring-pool
                    # tiles so the start is not tied to the gather-pool
                    # rotation (held by layer 1's trailing compute)
                    bor = borrow and g == 0
                    lo_tiles = []
                    g_lo = None
                    if not bor:
                        g_lo = gp.tile([128, cmax, d_hid], FP16, tag="g_lo")
                    p = 0
                    while p < call_lo[g]:
                        q = min(GMAX_NT, call_lo[g] - p)
                        if bor:
                            tg = "sc_g" if (p // GMAX_NT) % 2 == 0 else "sc_d"
                            t_ = scp.tile([128, SC_CALL // 128, d_hid], FP16,
                                          tag=tg)
                            dst = t_[:, :q // 128, :]
                            lo_tiles.append(t_)
                        else:
                            dst = g_lo[:, p // 128:(p + q) // 128, :]
                        nc.gpsimd.dma_gather(
                            dst, tab_lo[:],
                            gidx_sb[:, (lo_off + p) // 16:(lo_off + p + q) // 16],
                            q, nreg(q), d_hid)
                        p += q
                    g_hi = gp.tile([128, cmax, d_hid], FP16, tag="g_hi")
                    p = 0
                    while p < call_hi[g]:
                        q = min(GMAX_NT, call_hi[g] - p)
                        nc.gpsimd.dma_gather(
                            g_hi[:, p // 128:(p + q) // 128, :],
                            tab_hi[:],
                            gidx_sb[:, (hi_off + p) // 16:(hi_off + p + q) // 16],
                            q, nreg(q), d_hid)
                        p += q
                    wtile = sb.tile([128, SUP, d_hid], FP16, tag="wsup")
                    for j, b in enumerate(bs):
                        ncb = n_ch[b]
                        s_b = smp.tile([128, 128, nchmax], FP16, tag="s")
                        d_bc = dstl_sb[:, ch_off[b]:ch_off[b] + ncb] \
                            .unsqueeze(1).broadcast_to([128, 128, ncb])
                        nc.vector.tensor_tensor(
                            s_b[:, :, :ncb], d_bc, iota_sb[:, :, :ncb],
                            OP.is_equal)
                        o = ps.tile([128, d_hid], F32, tag="mm")
                        nlo = ncl[b]
                        for c in range(ncb):
                            if c < nlo:
                                gc = cb0[0][b] + c
                                if bor:
                                    rhs = lo_tiles[gc // 8][:, gc % 8, :]
                                else:
                                    rhs = g_lo[:, gc, :]
                            else:
                                rhs = g_hi[:, cb0[1][b] + (c - nlo), :]
                            nc.tensor.matmul(o[:], lhsT=s_b[:, :, c], rhs=rhs,
                                             start=(c == 0), stop=(c == ncb - 1))
                        post(b, o, wtile[:, j, :])
                    if sh_out:
                        b0, b1_ = bs[0], bs[-1] + 1
                        if b1_ <= lo_blk:
                            dst = sh_lo_t[128 * b0:128 * b1_, :]
                        else:
                            dst = sh_hi_t[128 * (b0 - lo_blk):
                                          128 * (b1_ - lo_blk), :]
                        nc.sync.dma_start(
                            dst.rearrange("(j p) f -> p j f", p=128),
                            wtile[:, :len(bs), :])
                    if hook is not None:
                        hook(g)

            # ---- P2+P3 fused: layer-1 aggregation -> h1T; GEMM2 -> hs2 ----
            def post1(b, o, wslot):
                tmp = sb.tile([128, d_hid], FP16, tag="tmp")
                nc.vector.tensor_scalar(tmp[:], o[:], dinv_sb[:, b:b + 1],
                                        None, OP.mult)
                for h in range(2):
                    tp = pst.tile([128, 128], FP16, tag="tps")
                    nc.tensor.transpose(tp[:], tmp[:, 128 * h:128 * (h + 1)],
                                        ident[:])
                    nc.scalar.activation(h1T[:, b, h, :], tp[:], AF.Relu,
                                         bias=b1c_sb[:, h:h + 1])
                g2 = ps.tile([128, d_hid], F32, tag="mm")
                for k in range(2):
                    nc.tensor.matmul(g2[:], lhsT=h1T[:, b, k, :],
                                     rhs=w2_sb[:, k, :],
                                     start=(k == 0), stop=(k == 1))
                nc.vector.tensor_scalar(wslot, g2[:], dinv_sb[:, b:b + 1],
                                        None, OP.mult)

            # ---- P5: layer-2 aggregation -> h2 shards ----
            def post2(b, o, wslot):
                nc.vector.scalar_tensor_tensor(
                    wslot, o[:], dinv_sb[:, b:b + 1], b2r_sb[:],
                    OP.mult, OP.add)

            # emit the lo-half AllGather as soon as blocks 0..lo_blk-1 are
            # written (mid-aggregation), overlapping it with the hi half
            ag_super = (lo_blk - 1) // SUP  # super index that completes lo

            def dump(sh_lo, sh_hi):
                nc.sync.dma_start(t_dbg[0:lo_loc, :], sh_lo[:])
                nc.sync.dma_start(t_dbg[lo_loc:n_loc, :], sh_hi[:])

            stop = False
            if debug_stage == 1:
                dump(hs1_sh_lo, hs1_sh_hi)
                stop = True
            if not stop:
                agg_layer(hs1_lo, hs1_hi, post1,
                          sh_out=(hs2_sh_lo, hs2_sh_hi),
                          hook=lambda g: all_gather(hs2_sh_lo, hs2_lo)
                          if g == ag_super else None)
                if debug_stage == 2:
                    dump(hs2_sh_lo, hs2_sh_hi)
                    stop = True
            if not stop:
                all_gather(hs2_sh_hi, hs2_hi)
                agg_layer(hs2_lo, hs2_hi, post2, borrow=True,
                          sh_out=(h2_sh_lo, h2_sh_hi),
                          hook=lambda g: all_gather(h2_sh_lo, h2_lo)
                          if g == ag_super else None)
                if debug_stage == 3:
                    dump(h2_sh_lo, h2_sh_hi)
                    stop = True
            if not stop:
                all_gather(h2_sh_hi, h2_hi)

            # ---- P7: edge scoring (edges-on-partitions; fused mult+accum) ----
            off = 0
            for (grp, csz) in (calls if not stop else []):
                s_tab = h2_hi if grp >= 2 else h2_lo
                d_tab = h2_hi if grp % 2 == 1 else h2_lo
                nch_sc = csz // 128
                gt = scp.tile([128, nch_sc, d_hid], FP16, tag="sc_g")
                nc.gpsimd.dma_gather(
                    gt[:], s_tab[:],
                    sidx_all[:, off // 16:(off + csz) // 16], csz, nreg(csz), d_hid)
                dt_ = scp.tile([128, nch_sc, d_hid], FP16, tag="sc_d")
                nc.gpsimd.dma_gather(
                    dt_[:], d_tab[:],
                    didx_all[:, off // 16:(off + csz) // 16], csz, nreg(csz), d_hid)
                sc_acc = sop.tile([128, nch_sc], F32, tag="scacc")
                for w in range(nch_sc):
                    junk = sb.tile([128, d_hid], FP16, tag="junk")
                    nc.vector.scalar_tensor_tensor(
                        junk[:], gt[:, w, :], 1.0, dt_[:, w, :],
                        OP.mult, OP.mult, accum_out=sc_acc[:, w:w + 1])
                sc_sb = sop.tile([128, nch_sc], F32, tag="scsb")
                nc.scalar.activation(sc_sb[:], sc_acc[:], AF.Sigmoid)
                nc.sync.dma_start(
                    t_out[:, off // 128:(off + csz) // 128], sc_sb[:])
                off += csz

    nc.compile()
    return nc


def _run(in_maps, cfg, meta, trace=False):
    nc = build_nc(cfg)
    res = bass_utils.run_bass_kernel_spmd(
        nc, in_maps, core_ids=list(range(cfg["n_cores"])), trace=trace)
    perm = meta["perm"]
    E, e_per = cfg["E"], cfg["e_per"]
    out = np.zeros(E, np.float32)
    for c in range(cfg["n_cores"]):
        # scores[p, ch] holds slot ch*128+p
        sc = np.asarray(res.results[c]["scores"], np.float32).T.reshape(-1)
        valid = perm[c] >= 0
        out[c * e_per + perm[c][valid]] = sc[valid]
    return out, res


def kernel(x, edge_index, W1, b1, W2, b2):
    in_maps, cfg, meta = prep_host(
        np.asarray(x), np.asarray(edge_index), np.asarray(W1),
        np.asarray(b1), np.asarray(W2), np.asarray(b2))
    out, _res = _run(in_maps, cfg, meta,
                     trace=bool(int(os.environ.get("KERNEL_TRACE", "0"))))
    return out


# revision 70
# speedup vs baseline: 1.0067x; 1.0034x over previous
"""Two-layer GCN + edge scoring on 8 Trainium2 NeuronCores.

Sharding: nodes row-sharded across cores (6250/core, padded to 6272 = 49
blocks of 128); aggregation edges partitioned by destination core and
grouped by destination block; weights replicated; three device-side
AllGathers move the per-node tables (hs1, hs2, h2) between phases.

Math trick: the GCN symmetric norm dinv[src]*dinv[dst] is separable, so
the gathered tables are pre-scaled by dinv (hs = dinv * (x@W)) and block
outputs post-scaled by dinv — the segment-sum masks stay pure 0/1 and the
scatter-add becomes S^T @ gathered_rows matmuls accumulated in PSUM.

Optimizations over the naive mask-streaming version (2.32ms -> 0.825ms):
- scatter masks generated ON DEVICE per block with one DVE is_equal op
  (dstl column stride-0-broadcast against an iota table) instead of
  streaming 64MB of host masks;
- dma_gather batched in 1024-index calls (HW ucode cap; larger calls
  fault the exec unit) to amortize the ~1us SWDGE descriptor-gen cost
  on the GPSIMD engine — the old 256-index calls made desc-gen the
  global bottleneck (85% Pool engine busy);
- edge scoring uses non-transpose gathers (edges on partitions) with a
  fused scalar_tensor_tensor multiply+accumulate per 128-edge chunk —
  no ones-matmul reduction, scores laid out [128, n/128];
- each table AllGather is split lo/hi (blocks 0-23 / 24-48, both
  sub-tables < 32768 rows for int16 indices): the lo collective fires
  mid-phase so lo-half gathers overlap the producer phase's tail;
- P0 loads x in 4-block batches and all shard writes are batched per
  super-group to cut HWDGE issue count;
- node -> (core, block) assignment is degree-balanced (greedy LPT) to
  shrink the shared max-over-cores aggregation bucket sizes;
- aggregation slots pack per (super-group, half) run with no per-block
  roundup: blocks' masks cover the union of chunks their rows can span
  across cores (straddle rows read dstl=PAD), cutting gather slots from
  124.5k to 109.4k per layer.
Engine occupancy (TimelineSim): DMA 96%, GPSIMD 81%, DVE 62%, PE 34%.
"""
import os
import sys

for p in ("/opt/trn_rl_repo", "/opt/pypackages"):
    if p not in sys.path:
        sys.path.insert(0, p)

import numpy as np

import concourse.bacc as bacc
import concourse.bass as bass
import concourse.mybir as mybir
import concourse.tile as tile
from concourse import bass_utils, library_config
from concourse.masks import make_identity

FP16 = mybir.dt.float16
F32 = mybir.dt.float32
I16 = mybir.dt.int16
AF = mybir.ActivationFunctionType
OP = mybir.AluOpType

NC_CORES = 8
D_IN = 512
D_HID = 256
SUP = 3             # dst blocks per aggregation super-group (one gather pair)
GMAX_NT = 1024      # max idx per non-transpose dma_gather (HW ucode limit)
SC_CALL = 1024      # edges per scoring gather call
PAD_DSTL = 999.0    # dstl pad value; never equals a block-local index


def _wrap_idx(idx, width):
    """int16 gather-index layout: [128, n/16], idx[i] at [i%16, i//16],
    replicated across the 8 groups of 16 partitions."""
    n = len(idx)
    assert n % 16 == 0
    t = np.asarray(idx, np.int16).reshape(n // 16, 16).T  # [16, n/16]
    out = np.tile(t, (8, 1))
    assert out.shape == (128, n // 16)
    if width > n // 16:
        out = np.concatenate(
            [out, np.zeros((128, width - n // 16), np.int16)], axis=1)
    return out


def _wrap_vals(vals, pad, n_slots):
    """Value layout matching non-transpose gather rows: slot i -> [i%128,
    i//128]. Returns [128, n_slots/128] float16."""
    a = np.full(n_slots, pad, np.float32)
    a[:len(vals)] = vals
    return a.reshape(n_slots // 128, 128).T.astype(np.float16)


def prep_host(x, edge_index, W1, b1, W2, b2, n_cores=NC_CORES):
    N, d_in = x.shape
    E = edge_index.shape[1]
    d_hid = W1.shape[1]
    n_per = N // n_cores
    nblk = (n_per + 127) // 128
    n_loc = nblk * 128
    NP = n_loc * n_cores
    # split each core's shard at block LO_BLK: the lo sub-shards AllGather
    # into a [n_cores*lo_loc] table as soon as the first LO_BLK blocks are
    # produced, overlapping the rest of the producing phase. Both tables
    # stay < 32768 rows for int16 gather indices.
    lo_blk = nblk // 2
    lo_loc = lo_blk * 128
    hi_loc = n_loc - lo_loc
    lo_n = lo_loc * n_cores
    hi_n = hi_loc * n_cores
    assert lo_n < 32768 and hi_n < 32768

    src0 = np.asarray(edge_index[0], np.int64)
    dst0 = np.asarray(edge_index[1], np.int64)
    src_f = np.concatenate([src0, np.arange(N, dtype=np.int64)])
    dst_f = np.concatenate([dst0, np.arange(N, dtype=np.int64)])

    deg = np.bincount(dst_f, minlength=N).astype(np.float64)
    dinv = np.where(deg > 0, 1.0 / np.sqrt(deg), 0.0).astype(np.float32)

    # ---- degree-balanced node -> (core, local slot) assignment ----
    # lo membership fixed up front (first lo_n nodes), then greedy LPT
    # packing equalizes each (core, block) bin's (in-deg-from-lo,
    # in-deg-from-hi) so the shared bucket sizes max_c cnt shrink to ~mean.
    n_lo_real = min(lo_n, N)
    node_is_lo = np.zeros(N, bool)
    node_is_lo[:n_lo_real] = True
    src_is_lo = node_is_lo[src_f]
    dl = np.bincount(dst_f[src_is_lo], minlength=N)     # in-deg from lo srcs
    dh = np.bincount(dst_f[~src_is_lo], minlength=N)    # in-deg from hi srcs

    node_core = np.zeros(N, np.int64)
    node_j = np.zeros(N, np.int64)

    def pack(nodes, nbins_per_core, cap_real, j_base):
        nbins = n_cores * nbins_per_core
        s_lo = np.zeros(nbins)
        s_hi = np.zeros(nbins)
        fill = np.zeros(nbins, np.int64)
        core_fill = np.zeros(n_cores, np.int64)
        order_ = np.argsort(-(dl[nodes] + dh[nodes]))
        for v in nodes[order_]:
            score = np.maximum(s_lo + dl[v], s_hi + dh[v]) + \
                (s_lo + dl[v]) + (s_hi + dh[v])
            bad = (fill >= 128) | \
                (core_fill[np.arange(nbins) // nbins_per_core] >= cap_real)
            score[bad] = np.inf
            k = int(np.argmin(score))
            s_lo[k] += dl[v]
            s_hi[k] += dh[v]
            c = k // nbins_per_core
            node_core[v] = c
            node_j[v] = j_base + (k % nbins_per_core) * 128 + fill[k]
            fill[k] += 1
            core_fill[c] += 1

    lo_nodes = np.nonzero(node_is_lo)[0]
    hi_nodes = np.nonzero(~node_is_lo)[0]
    pack(lo_nodes, lo_blk, lo_loc, 0)
    pack(hi_nodes, nblk - lo_blk, n_per - lo_loc, lo_loc)

    # node -> padded slot tables
    node_at = np.full((n_cores, n_loc), -1, np.int64)
    node_at[node_core, node_j] = np.arange(N)

    d_core = node_core[dst_f]
    d_j = node_j[dst_f]
    blk = d_j // 128
    dstl = d_j % 128
    core_of = d_core
    s_core = node_core[src_f]
    s_j = node_j[src_f]
    is_lo = s_j < lo_loc
    # row in the lo/hi sub-table
    rel = np.where(is_lo, s_core * lo_loc + s_j,
                   s_core * hi_loc + (s_j - lo_loc))

    # bucket aggregation edges (incl. self-loops) by (core, block, lo/hi)
    hi_f = (~is_lo).astype(np.int64)
    key = (core_of * nblk + blk) * 2 + hi_f  # bucket id
    order = np.lexsort((rel, key))
    key_s = key[order]
    rel_s = rel[order]
    dstl_s = dstl[order]
    nbuck = n_cores * nblk * 2
    counts = np.bincount(key_s, minlength=nbuck)
    starts = np.concatenate([[0], np.cumsum(counts)])

    def bucket(c, b, h):
        k = (c * nblk + b) * 2 + h
        s, e = starts[k], starts[k + 1]
        return rel_s[s:e], dstl_s[s:e]

    cnt = counts.reshape(n_cores, nblk, 2)

    # super-groups of SUP blocks -> one lo + one hi gather run each.
    # Slots pack per (super, half) with NO per-block padding (the run pads
    # to 128 only at its end, max over cores). Each block's mask covers
    # the union over cores of the chunks its rows can occupy; rows of
    # neighboring blocks inside straddle chunks read dstl=PAD so they
    # never match. Degree balancing keeps the unions tight.
    n_sup = (nblk + SUP - 1) // SUP
    supers = [list(range(g * SUP, min((g + 1) * SUP, nblk)))
              for g in range(n_sup)]
    call_lo, call_hi = [], []
    cb0 = np.zeros((2, nblk), np.int64)  # union chunk range per (half, blk)
    cb1 = np.zeros((2, nblk), np.int64)
    for g, bs in enumerate(supers):
        for h in range(2):
            start = np.zeros((n_cores, len(bs) + 1), np.int64)
            for j, b in enumerate(bs):
                start[:, j + 1] = start[:, j] + cnt[:, b, h]
            run = ((int(start[:, -1].max()) + 127) // 128) * 128
            (call_lo if h == 0 else call_hi).append(run)
            for j, b in enumerate(bs):
                cb0[h, b] = int(start[:, j].min()) // 128
                cb1[h, b] = max((int(start[:, j + 1].max()) + 127) // 128,
                                cb0[h, b] + 1)
    cmax = max(max(call_lo), max(call_hi)) // 128  # chunks per gather tile
    tot_agg = int(sum(call_lo) + sum(call_hi))

    ncl = cb1[0] - cb0[0]                # lo mask chunks per block
    n_ch = ncl + (cb1[1] - cb0[1])       # total mask chunks per block
    nchmax = int(n_ch.max())
    ch_off = np.concatenate([[0], np.cumsum(n_ch)])  # dstl col offsets

    # per-core agg gather indices + per-block dstl columns
    gidx = np.zeros((n_cores, 128, tot_agg // 16), np.int16)
    dstl_t = np.full((n_cores, 128, int(ch_off[-1])), PAD_DSTL, np.float16)
    for c in range(n_cores):
        off = 0
        for g, bs in enumerate(supers):
            for h in range(2):
                run = call_lo[g] if h == 0 else call_hi[g]
                ia = np.zeros(run, np.int64)
                dv = np.full(run, PAD_DSTL, np.float32)
                p = 0
                pos = []
                for b in bs:
                    brel, bdv = bucket(c, b, h)
                    ia[p:p + len(brel)] = brel
                    dv[p:p + len(brel)] = bdv
                    pos.append((p, p + len(brel)))
                    p += len(brel)
                gidx[c, :, off // 16: (off + run) // 16] = \
                    _wrap_idx(ia, run // 16)
                wrapped = dv.reshape(run // 128, 128).T  # [128, chunks]
                for j, b in enumerate(bs):
                    s, e = pos[j]
                    c0, c1 = int(cb0[h, b]), int(cb1[h, b])
                    seg = wrapped[:, c0:c1].astype(np.float32).copy()
                    slot = (np.arange(c0 * 128, c1 * 128)
                            .reshape(c1 - c0, 128).T)
                    seg[(slot < s) | (slot >= e)] = PAD_DSTL
                    co = ch_off[b] + (0 if h == 0 else ncl[b])
                    dstl_t[c, :, co:co + (c1 - c0)] = seg.astype(np.float16)
                off += run
        assert off == tot_agg

    # iota table for mask generation: iota_rep[p, d, c] = d
    iota_rep = np.tile(
        np.arange(128, dtype=np.float16)[None, :, None], (128, 1, nchmax))

    # scoring: original E edges, n_cores contiguous slices, 4-way grouped
    # by (src-half, dst-half). Group 0 (lo,lo) only needs the lo AllGather.
    e_per = E // n_cores
    sc_src = rel[:E]
    sc_dst = np.where(d_j[:E] < lo_loc, d_core[:E] * lo_loc + d_j[:E],
                      d_core[:E] * hi_loc + (d_j[:E] - lo_loc))
    s_hi = (~is_lo[:E]).astype(np.int64)
    d_hi = (d_j[:E] >= lo_loc).astype(np.int64)
    groups = [[None] * 4 for _ in range(n_cores)]
    for c in range(n_cores):
        sl_ = slice(c * e_per, (c + 1) * e_per)
        gg = s_hi[sl_] * 2 + d_hi[sl_]
        for g in range(4):
            groups[c][g] = np.nonzero(gg == g)[0]
    g_sz = [max(len(groups[c][g]) for c in range(n_cores)) for g in range(4)]
    g_sz = [((s + SC_CALL - 1) // SC_CALL) * SC_CALL for s in g_sz]
    tot_sc = sum(g_sz)
    calls = []  # (group, call_size)
    for g in range(4):
        for _ in range(g_sz[g] // SC_CALL):
            calls.append((g, SC_CALL))

    sidx = np.zeros((n_cores, 128, tot_sc // 16), np.int16)
    didx = np.zeros((n_cores, 128, tot_sc // 16), np.int16)
    perm = np.full((n_cores, tot_sc), -1, np.int64)
    for c in range(n_cores):
        off = 0
        for g in range(4):
            idxs = groups[c][g]
            ss = sc_src[c * e_per + idxs]
            dd = sc_dst[c * e_per + idxs]
            sa = np.zeros(g_sz[g], np.int64)
            da = np.zeros(g_sz[g], np.int64)
            sa[:len(idxs)] = ss
            da[:len(idxs)] = dd
            perm[c, off:off + len(idxs)] = idxs
            p = 0
            while p < g_sz[g]:
                s = min(SC_CALL, g_sz[g] - p)
                sidx[c, :, (off + p) // 16:(off + p + s) // 16] = \
                    _wrap_idx(sa[p:p + s], s // 16)
                didx[c, :, (off + p) // 16:(off + p + s) // 16] = \
                    _wrap_idx(da[p:p + s], s // 16)
                p += s
            off += g_sz[g]

    # dense per-core tensors
    W1h = np.asarray(W1, np.float32).reshape(4, 128, d_hid).transpose(1, 0, 2)
    W2h = np.asarray(W2, np.float32).reshape(2, 128, d_hid).transpose(1, 0, 2)
    b1c = np.asarray(b1, np.float32).reshape(2, 128).T.copy()  # [128, 2]
    b2r = np.tile(np.asarray(b2, np.float32)[None, :], (128, 1))  # [128, 256]

    xf = np.asarray(x, np.float32)
    in_maps = []
    for c in range(n_cores):
        sel = node_at[c]  # padded slot -> node id (-1 = empty)
        xs = np.zeros((n_loc, d_in), np.float32)
        xs[sel >= 0] = xf[sel[sel >= 0]]
        xt = np.ascontiguousarray(
            xs.T.reshape(4, 128, nblk, 128).transpose(1, 2, 0, 3))
        dv = np.zeros(n_loc, np.float32)
        dv[sel >= 0] = dinv[sel[sel >= 0]]
        dcol = dv.reshape(nblk, 128).T.copy()  # [128, nblk]
        in_maps.append({
            "xt": xt.astype(np.float16),
            "w1": W1h.astype(np.float16),
            "w2": W2h.astype(np.float16),
            "b1c": b1c, "b2r": b2r,
            "dinv": dcol,
            "gidx": gidx[c],
            "dstl": dstl_t[c],
            "iota": iota_rep,
            "sidx": sidx[c], "didx": didx[c],
        })

    cfg = dict(n_cores=n_cores, N=N, E=E, d_in=d_in, d_hid=d_hid,
               n_per=n_per, nblk=nblk, n_loc=n_loc, NP=NP,
               lo_blk=lo_blk, lo_loc=lo_loc, hi_loc=hi_loc,
               lo_n=lo_n, hi_n=hi_n,
               n_ch=n_ch.tolist(), ncl=ncl.tolist(),
               cb0=cb0.tolist(), cb1=cb1.tolist(),
               nchmax=nchmax, ch_off=ch_off.tolist(),
               supers=supers, call_lo=call_lo, call_hi=call_hi, cmax=cmax,
               tot_agg=tot_agg, g_sz=g_sz, tot_sc=tot_sc, calls=calls,
               e_per=e_per)
    meta = dict(perm=perm, node_at=node_at)
    return in_maps, cfg, meta


def build_nc(cfg, timing_mode=False, debug_stage=0):
    """debug_stage: 0=full, 1=stop after GEMM1 (dump hs1), 2=stop after
    layer-1 agg + GEMM2 (dump hs2), 3=stop after layer-2 agg (dump h2)."""
    n_cores = cfg["n_cores"]
    d_hid = cfg["d_hid"]
    nblk, n_loc, NP = cfg["nblk"], cfg["n_loc"], cfg["NP"]
    lo_blk, lo_loc, hi_loc = cfg["lo_blk"], cfg["lo_loc"], cfg["hi_loc"]
    lo_n, hi_n = cfg["lo_n"], cfg["hi_n"]
    n_ch, ncl = cfg["n_ch"], cfg["ncl"]
    cb0, cb1 = cfg["cb0"], cfg["cb1"]
    nchmax, ch_off = cfg["nchmax"], cfg["ch_off"]
    supers, call_lo, call_hi = cfg["supers"], cfg["call_lo"], cfg["call_hi"]
    cmax = cfg["cmax"]
    tot_agg, tot_sc, calls = cfg["tot_agg"], cfg["tot_sc"], cfg["calls"]

    nc = bacc.Bacc("TRN2", target_bir_lowering=False, debug=False,
                   num_devices=1 if timing_mode else n_cores)

    def all_gather(shard, full):
        if timing_mode:
            # timing workalike: local copy stands in for the collective;
            # real AG cost (6 half-table AGs ~13us each) added by caller
            nc.sync.dma_start(full[0:shard.shape[0], :], shard[:])
        else:
            nc.gpsimd.collective_compute(
                "AllGather", OP.bypass, replica_groups=rg,
                ins=[shard.opt()], outs=[full.opt()])

    t_xt = nc.dram_tensor("xt", [128, nblk, 4, 128], FP16, kind="ExternalInput").ap()
    t_w1 = nc.dram_tensor("w1", [128, 4, d_hid], FP16, kind="ExternalInput").ap()
    t_w2 = nc.dram_tensor("w2", [128, 2, d_hid], FP16, kind="ExternalInput").ap()
    t_b1c = nc.dram_tensor("b1c", [128, 2], F32, kind="ExternalInput").ap()
    t_b2r = nc.dram_tensor("b2r", [128, d_hid], F32, kind="ExternalInput").ap()
    t_dinv = nc.dram_tensor("dinv", [128, nblk], F32, kind="ExternalInput").ap()
    t_gidx = nc.dram_tensor("gidx", [128, tot_agg // 16], I16,
                            kind="ExternalInput").ap()
    t_dstl = nc.dram_tensor("dstl", [128, ch_off[-1]], FP16,
                            kind="ExternalInput").ap()
    t_iota = nc.dram_tensor("iota", [128, 128, nchmax], FP16,
                            kind="ExternalInput").ap()
    t_sidx = nc.dram_tensor("sidx", [128, tot_sc // 16], I16,
                            kind="ExternalInput").ap()
    t_didx = nc.dram_tensor("didx", [128, tot_sc // 16], I16,
                            kind="ExternalInput").ap()
    t_out = nc.dram_tensor("scores", [128, tot_sc // 128], F32,
                           kind="ExternalOutput").ap()
    t_dbg = None
    if debug_stage:
        t_dbg = nc.dram_tensor("dbg", [n_loc, d_hid], FP16,
                               kind="ExternalOutput").ap()

    rg = [list(range(n_cores))]

    with tile.TileContext(nc) as tc:
        with (
            tc.tile_pool(name="const", bufs=1) as cp,
            tc.tile_pool(name="xtp", bufs=4) as xtp,
            tc.tile_pool(name="sbuf", bufs=3) as sb,
            tc.tile_pool(name="gath", bufs=2) as gp,
            tc.tile_pool(name="smask", bufs=2) as smp,
            tc.tile_pool(name="scg", bufs=3) as scp,
            tc.tile_pool(name="scs", bufs=2) as sop,
            tc.tile_pool(name="psum", bufs=3, space="PSUM") as ps,
            tc.tile_pool(name="psum_t", bufs=2, space="PSUM") as pst,
            tc.tile_pool(name="dram", bufs=1, space="DRAM") as dr,
        ):
            nc.gpsimd.load_library(library_config.mlp)

            # ---- resident constants ----
            # only w1 + dinv block the first GEMM; the bulky index tables
            # stream during P0's DMA slack (emitted after the first xt load)
            w1_sb = cp.tile([128, 4, d_hid], FP16)
            nc.sync.dma_start(w1_sb[:], t_w1)
            dinv_sb = cp.tile([128, nblk], F32)
            nc.sync.dma_start(dinv_sb[:], t_dinv)
            w2_sb = cp.tile([128, 2, d_hid], FP16)
            b1c_sb = cp.tile([128, 2], F32)
            b2r_sb = cp.tile([128, d_hid], F32)
            gidx_sb = cp.tile([128, tot_agg // 16], I16)
            dstl_sb = cp.tile([128, ch_off[-1]], FP16)
            iota_sb = cp.tile([128, 128, nchmax], FP16)
            ident = cp.tile([128, 128], FP16)
            h1T = cp.tile([128, nblk, 2, 128], FP16)
            sidx_all = cp.tile([128, tot_sc // 16], I16)
            didx_all = cp.tile([128, tot_sc // 16], I16)

            _regs = {}

            def nreg(n):
                if n not in _regs:
                    _regs[n] = nc.gpsimd.to_reg(n)
                return _regs[n]

            def late_consts():
                nc.sync.dma_start(gidx_sb[:], t_gidx)
                nc.sync.dma_start(dstl_sb[:], t_dstl)
                nc.sync.dma_start(iota_sb[:], t_iota)
                nc.sync.dma_start(w2_sb[:], t_w2)
                nc.sync.dma_start(b1c_sb[:], t_b1c)
                nc.sync.dma_start(b2r_sb[:], t_b2r)
                nc.sync.dma_start(sidx_all[:], t_sidx)
                nc.sync.dma_start(didx_all[:], t_didx)
                make_identity(nc, ident[:])

            # ---- DRAM tables (lo/hi split for phased AllGathers) ----
            hs1_sh_lo = dr.tile([lo_loc, d_hid], FP16)
            hs1_sh_hi = dr.tile([hi_loc, d_hid], FP16)
            hs1_lo = dr.tile([lo_n, d_hid], FP16)
            hs1_hi = dr.tile([hi_n, d_hid], FP16)
            hs2_sh_lo = dr.tile([lo_loc, d_hid], FP16)
            hs2_sh_hi = dr.tile([hi_loc, d_hid], FP16)
            hs2_lo = dr.tile([lo_n, d_hid], FP16)
            hs2_hi = dr.tile([hi_n, d_hid], FP16)
            h2_sh_lo = dr.tile([lo_loc, d_hid], FP16)
            h2_sh_hi = dr.tile([hi_loc, d_hid], FP16)
            h2_lo = dr.tile([lo_n, d_hid], FP16)
            h2_hi = dr.tile([hi_n, d_hid], FP16)

            # ---- P0: GEMM1 + dinv scale -> hs1 shards; AG1a mid-phase ----
            # 4-block batches cut HWDGE issue count 4x (aligned with the
            # lo/hi boundary at lo_blk=24)
            for q in range((nblk + 3) // 4):
                bs4 = list(range(4 * q, min(4 * q + 4, nblk)))
                xt_b = xtp.tile([128, 4, 4, 128], FP16, tag="xtb")
                nc.sync.dma_start(xt_b[:, :len(bs4)],
                                  t_xt[:, bs4[0]:bs4[-1] + 1, :, :])
                if q == 0:
                    late_consts()
                hs4 = sb.tile([128, 4, d_hid], FP16, tag="hs4")
                for j, b in enumerate(bs4):
                    g1 = ps.tile([128, d_hid], F32, tag="mm")
                    for k in range(4):
                        nc.tensor.matmul(
                            g1[:], lhsT=xt_b[:, j, k, :],
                            rhs=w1_sb[:, k, :], start=(k == 0), stop=(k == 3))
                    nc.vector.tensor_scalar(hs4[:, j, :], g1[:],
                                            dinv_sb[:, b:b + 1], None, OP.mult)
                b0, b1_ = bs4[0], bs4[-1] + 1
                if b1_ <= lo_blk:
                    dst = hs1_sh_lo[128 * b0:128 * b1_, :]
                else:
                    dst = hs1_sh_hi[128 * (b0 - lo_blk):128 * (b1_ - lo_blk), :]
                nc.sync.dma_start(
                    dst.rearrange("(j p) f -> p j f", p=128), hs4[:, :len(bs4)])
                if b1_ == lo_blk:
                    all_gather(hs1_sh_lo, hs1_lo)
            all_gather(hs1_sh_hi, hs1_hi)

            # ---- aggregation layer: per super-group gather, per block mask
            #      gen + scatter matmuls; post(b, o_psum) consumes output ----
            def agg_layer(tab_lo, tab_hi, post, sh_out=None, hook=None,
                          borrow=False):
                # post(b, o, wslot) fills wslot with the block's [128, d_hid]
                # fp16 result; results batch-write per super into sh_out
                sh_lo_t, sh_hi_t = sh_out if sh_out else (None, None)
                off = 0
                offs = []
                for g in range(len(supers)):
                    offs.append((off, off + call_lo[g]))
                    off += call_lo[g] + call_hi[g]
                for g, bs in enumerate(supers):
                    lo_off, hi_off = offs[g]
                    # first super of layer 2: gather into idle scoring-pool
                    # tiles so the start is not tied to the gather-pool
                    # rotation (held by layer 1's trailing compute)
                    bor = borrow and g == 0
                    lo_tiles = []
                    g_lo = None
                    if not bor:
                        g_lo = gp.tile([128, cmax, d_hid], FP16, tag="g_lo")
                    p = 0
                    while p < call_lo[g]:
                        q = min(GMAX_NT, call_lo[g] - p)
                        if bor:
                            tg = "sc_g" if (p // GMAX_NT) % 2 == 0 else "sc_d"
                            t_ = scp.tile([128, SC_CALL // 128, d_hid], FP16,
                                          tag=tg)
                            dst = t_[:, :q // 128, :]
                            lo_tiles.append(t_)
                        else:
                            dst = g_lo[:, p // 128:(p + q) // 128, :]
                        nc.gpsimd.dma_gather(
                            dst, tab_lo[:],
                            gidx_sb[:, (lo_off + p) // 16:(lo_off + p + q) // 16],
                            q, nreg(q), d_hid)
                        p += q
                    g_hi = gp.tile([128, cmax, d_hid], FP16, tag="g_hi")
                    p = 0
                    while p < call_hi[g]:
                        q = min(GMAX_NT, call_hi[g] - p)
                        nc.gpsimd.dma_gather(
                            g_hi[:, p // 128:(p + q) // 128, :],
                            tab_hi[:],
                            gidx_sb[:, (hi_off + p) // 16:(hi_off + p + q) // 16],
                            q, nreg(q), d_hid)
                        p += q
                    wtile = sb.tile([128, SUP, d_hid], FP16, tag="wsup")
                    for j, b in enumerate(bs):
                        ncb = n_ch[b]
                        s_b = smp.tile([128, 128, nchmax], FP16, tag="s")
                        d_bc = dstl_sb[:, ch_off[b]:ch_off[b] + ncb] \
                            .unsqueeze(1).broadcast_to([128, 128, ncb])
                        nc.vector.tensor_tensor(
                            s_b[:, :, :ncb], d_bc, iota_sb[:, :, :ncb],
                            OP.is_equal)
                        o = ps.tile([128, d_hid], F32, tag="mm")
                        nlo = ncl[b]
                        for c in range(ncb):
                            if c < nlo:
                                gc = cb0[0][b] + c
                                if bor:
                                    rhs = lo_tiles[gc // 8][:, gc % 8, :]
                                else:
                                    rhs = g_lo[:, gc, :]
                            else:
                                rhs = g_hi[:, cb0[1][b] + (c - nlo), :]
                            nc.tensor.matmul(o[:], lhsT=s_b[:, :, c], rhs=rhs,
                                             start=(c == 0), stop=(c == ncb - 1))
                        post(b, o, wtile[:, j, :])
                    if sh_out:
                        b0, b1_ = bs[0], bs[-1] + 1
                        if b1_ <= lo_blk:
                            dst = sh_lo_t[128 * b0:128 * b1_, :]
                        else:
                            dst = sh_hi_t[128 * (b0 - lo_blk):
                                          128 * (b1_ - lo_blk), :]
                        nc.sync.dma_start(
                            dst.rearrange("(j p) f -> p j f", p=128),
                            wtile[:, :len(bs), :])
                    if hook is not None:
                        hook(g)

            # ---- P2+P3 fused: layer-1 aggregation -> h1T; GEMM2 -> hs2 ----
            def post1(b, o, wslot):
                tmp = sb.tile([128, d_hid], FP16, tag="tmp")
                nc.vector.tensor_scalar(tmp[:], o[:], dinv_sb[:, b:b + 1],
                                        None, OP.mult)
                for h in range(2):
                    tp = pst.tile([128, 128], FP16, tag="tps")
                    nc.tensor.transpose(tp[:], tmp[:, 128 * h:128 * (h + 1)],
                                        ident[:])
                    nc.scalar.activation(h1T[:, b, h, :], tp[:], AF.Relu,
                                         bias=b1c_sb[:, h:h + 1])
                g2 = ps.tile([128, d_hid], F32, tag="mm")
                for k in range(2):
                    nc.tensor.matmul(g2[:], lhsT=h1T[:, b, k, :],
                                     rhs=w2_sb[:, k, :],
                                     start=(k == 0), stop=(k == 1))
                nc.vector.tensor_scalar(wslot, g2[:], dinv_sb[:, b:b + 1],
                                        None, OP.mult)

            # ---- P5: layer-2 aggregation -> h2 shards ----
            def post2(b, o, wslot):
                nc.vector.scalar_tensor_tensor(
                    wslot, o[:], dinv_sb[:, b:b + 1], b2r_sb[:],
                    OP.mult, OP.add)

            # emit the lo-half AllGather as soon as blocks 0..lo_blk-1 are
            # written (mid-aggregation), overlapping it with the hi half
            ag_super = (lo_blk - 1) // SUP  # super index that completes lo

            def dump(sh_lo, sh_hi):
                nc.sync.dma_start(t_dbg[0:lo_loc, :], sh_lo[:])
                nc.sync.dma_start(t_dbg[lo_loc:n_loc, :], sh_hi[:])

            stop = False
            if debug_stage == 1:
                dump(hs1_sh_lo, hs1_sh_hi)
                stop = True
            if not stop:
                agg_layer(hs1_lo, hs1_hi, post1,
                          sh_out=(hs2_sh_lo, hs2_sh_hi),
                          hook=lambda g: all_gather(hs2_sh_lo, hs2_lo)
                          if g == ag_super else None)
                if debug_stage == 2:
                    dump(hs2_sh_lo, hs2_sh_hi)
                    stop = True
            if not stop:
                all_gather(hs2_sh_hi, hs2_hi)
                agg_layer(hs2_lo, hs2_hi, post2, borrow=True,
                          sh_out=(h2_sh_lo, h2_sh_hi),
                          hook=lambda g: all_gather(h2_sh_lo, h2_lo)
                          if g == ag_super else None)
                if debug_stage == 3:
                    dump(h2_sh_lo, h2_sh_hi)
                    stop = True
            if not stop:
                all_gather(h2_sh_hi, h2_hi)

            # ---- P7: edge scoring (edges-on-partitions; fused mult+accum) ----
            off = 0
            for (grp, csz) in (calls if not stop else []):
                s_tab = h2_hi if grp >= 2 else h2_lo
                d_tab = h2_hi if grp % 2 == 1 else h2_lo
                nch_sc = csz // 128
                gt = scp.tile([128, nch_sc, d_hid], FP16, tag="sc_g")
                nc.gpsimd.dma_gather(
                    gt[:], s_tab[:],
                    sidx_all[:, off // 16:(off + csz) // 16], csz, nreg(csz), d_hid)
                dt_ = scp.tile([128, nch_sc, d_hid], FP16, tag="sc_d")
                nc.gpsimd.dma_gather(
                    dt_[:], d_tab[:],
                    didx_all[:, off // 16:(off + csz) // 16], csz, nreg(csz), d_hid)
                sc_acc = sop.tile([128, nch_sc], F32, tag="scacc")
                for w in range(nch_sc):
                    junk = sb.tile([128, d_hid], FP16, tag="junk")
                    nc.vector.scalar_tensor_tensor(
                        junk[:], gt[:, w, :], 1.0, dt_[:, w, :],
                        OP.mult, OP.mult, accum_out=sc_acc[:, w:w + 1])
                sc_sb = sop.tile([128, nch_sc], F32, tag="scsb")
                nc.scalar.activation(sc_sb[:], sc_acc[:], AF.Sigmoid)
                nc.sync.dma_start(
                    t_out[:, off // 128:(off + csz) // 128], sc_sb[:])
                off += csz

    nc.compile()
    return nc


def _run(in_maps, cfg, meta, trace=False):
    nc = build_nc(cfg)
    res = bass_utils.run_bass_kernel_spmd(
        nc, in_maps, core_ids=list(range(cfg["n_cores"])), trace=trace)
    perm = meta["perm"]
    E, e_per = cfg["E"], cfg["e_per"]
    out = np.zeros(E, np.float32)
    for c in range(cfg["n_cores"]):
        # scores[p, ch] holds slot ch*128+p
        sc = np.asarray(res.results[c]["scores"], np.float32).T.reshape(-1)
        valid = perm[c] >= 0
        out[c * e_per + perm[c][valid]] = sc[valid]
    return out, res


def kernel(x, edge_index, W1, b1, W2, b2):
    in_maps, cfg, meta = prep_host(
        np.asarray(x), np.asarray(edge_index), np.asarray(W1),
        np.asarray(b1), np.asarray(W2), np.asarray(b2))
    out, _res = _run(in_maps, cfg, meta,
                     trace=bool(int(os.environ.get("KERNEL_TRACE", "0"))))
    return out
